# revision 48
# baseline (speedup 1.0000x reference)
"""GQA attention kernel for 8 TRN2 NeuronCores (Bass/Tile) — v3.

Sharding: tokens sharded 8 ways (2 batches x 4 chunks of 512).  Each core
computes Q/K/V projections for its 512 tokens in a transposed
(feature-on-partition) layout, all-gathers K/V within its 4-core batch
group, then runs attention with scores in [k_tok, q_tok] layout and a
ones column appended to V so the softmax denominator falls out of the PV
matmul.  Output token rows are disjoint per core -> no collective for O.

v3 schedule: one fused software-pipelined stream.  Lead-in computes
K proj j=0,1 (j-granular gathers so attention can start ~35us in),
V proj (+ split V gather), Q proj m=0,1.  Then a 128-step attention
stream (one step per (m, kp)) keeps the scalar engine (exp) saturated;
PE slack in each step is filled with the remaining K projections
(blocks 0-5) and just-in-time Q projections (m+2 per block).  PV lags
LAG steps behind scores/exp and consumes KV chunks in gather-arrival
order (even kp first).  O projection runs as a tail with Wo streamed
in per-nb waves.

RoPE pair-swap is 4 contiguous-partition SBUF->SBUF DMA copies instead
of a permutation matmul: Wq/Wk columns are host-permuted so each 64-row
head-half holds [evens | odds]; bf16 cos/sin tables (half the DMA bytes
of f32) are built to match.  Softmax denominators are inverted with the
fast fp32 reciprocal approximation (~5x faster than nc.vector.reciprocal).

All matmuls bf16 (fp32 is 4x slower on the PE); fp32 PSUM accum.
PSUM budget: proj 2 banks + scores 2x2 banks + PV 2 banks = 8.
"""
import numpy as np
import ml_dtypes

D_MODEL = 2048
KV_DIM = 1024
B = 2
S = 2048
SC = 512            # tokens per core
N_CORES = 8
ROPE_BASE = 10000.0
BF16 = ml_dtypes.bfloat16

_cache = {}


def _host_prep():
    if "perm" in _cache:
        return
    # head-pairing permutation (as v2): row chunk m pairs the two query
    # heads whose KV-head halves share a K chunk.
    perm = np.zeros(D_MODEL, dtype=np.int64)
    for g in range(16):
        for qi in range(2):
            for d in range(64):
                f = g * 128 + qi * 64 + d
                p = ((g // 2) * 2 + qi) * 128 + (g % 2) * 64 + d
                perm[p] = f
    _cache["perm"] = perm  # Wo rows keep this (attnout dims are not eo-split)

    # evens-then-odds split within each 64-row head-half, so the rope
    # pair-swap becomes two contiguous 32-partition block swaps.
    eo = np.concatenate([np.arange(0, 64, 2), np.arange(1, 64, 2)])
    qperm = np.zeros(D_MODEL, dtype=np.int64)
    for b64 in range(D_MODEL // 64):
        qperm[64 * b64:64 * (b64 + 1)] = perm[64 * b64 + eo]
    _cache["qperm"] = qperm
    kperm = np.zeros(KV_DIM, dtype=np.int64)
    for b64 in range(KV_DIM // 64):
        kperm[64 * b64:64 * (b64 + 1)] = 64 * b64 + eo
    _cache["kperm"] = kperm

    theta = ROPE_BASE ** (-np.arange(1024, dtype=np.float64) / 1024.0)
    tabs = []
    for ci in range(4):
        pos = np.arange(ci * SC, (ci + 1) * SC, dtype=np.float64)
        tab = np.zeros((24, 128, 2 * SC), dtype=np.float64)
        for c in range(24):
            if c < 8:
                flat = kperm[np.arange(128 * c, 128 * (c + 1))]
            else:
                flat = qperm[128 * (c - 8):128 * (c - 7)]
            ang = theta[flat // 2][:, None] * pos[None, :]
            sign = np.where(flat % 2 == 0, -1.0, 1.0)
            tab[c, :, :SC] = np.cos(ang)
            tab[c, :, SC:] = sign[:, None] * np.sin(ang)
        tabs.append(tab.astype(BF16))
    _cache["tabs"] = tabs

    # 32-row block-swap permutation (evens block <-> odds block per head-half)
    Pswap = np.zeros((128, 128), dtype=np.float32)
    for h in range(2):
        for b in range(32):
            Pswap[64 * h + 32 + b, 64 * h + b] = 1.0
            Pswap[64 * h + b, 64 * h + 32 + b] = 1.0
    _cache["Pswap"] = Pswap.astype(BF16)


def _build_nc():
    if "nc" in _cache:
        return _cache["nc"]
    import concourse.bacc as bacc
    import concourse.mybir as mybir
    import concourse.tile as tile

    f32 = mybir.dt.float32
    bf16 = mybir.dt.bfloat16
    Exp = mybir.ActivationFunctionType.Exp
    mult = mybir.AluOpType.mult
    add = mybir.AluOpType.add

    nc = bacc.Bacc("TRN2", target_bir_lowering=False, debug=False,
                   num_devices=N_CORES)

    # xT/Wq/Wk are host-tiled so every SBUF load is one contiguous
    # full-rate DMA ([128, 2048] per chunk; the strided/rearranged loads
    # measured ~82GB/s and clogged the queues)
    xT_in = nc.dram_tensor("xT", [128, 16 * SC], bf16, kind="ExternalInput").ap()
    Wq_in = nc.dram_tensor("Wq", [16, 128, D_MODEL], bf16, kind="ExternalInput").ap()
    Wk_in = nc.dram_tensor("Wk", [8, 128, D_MODEL], bf16, kind="ExternalInput").ap()
    Wv_in = nc.dram_tensor("Wv", [D_MODEL, KV_DIM], bf16, kind="ExternalInput").ap()
    Wo_in = nc.dram_tensor("Wo", [D_MODEL, D_MODEL], bf16, kind="ExternalInput").ap()
    rt_in = nc.dram_tensor("ropetab", [24, 128, 2 * SC], bf16,
                           kind="ExternalInput").ap()
    psw_in = nc.dram_tensor("Pswap", [128, 128], bf16, kind="ExternalInput").ap()
    out_dram = nc.dram_tensor("out", [SC, D_MODEL], f32, kind="ExternalOutput").ap()

    GROUPS = [[0, 1, 2, 3], [4, 5, 6, 7]]
    LAG = 6

    from contextlib import ExitStack

    with tile.TileContext(nc) as tc, nc.allow_low_precision(reason="bf16 matmul pipeline by design"):
        with ExitStack() as ostack:
            dram = ostack.enter_context(tc.tile_pool(name="dram", bufs=1, space="DRAM"))
            persist = ostack.enter_context(tc.tile_pool(name="persist", bufs=1))
            kfp = ostack.enter_context(tc.tile_pool(name="kfp", bufs=5))   # Kfull rotation
            vfp = ostack.enter_context(tc.tile_pool(name="vfp", bufs=1))   # Vfull (static)
            wop = ostack.enter_context(tc.tile_pool(name="wop", bufs=2))   # Wv halves + Wo waves
            qrp = ostack.enter_context(tc.tile_pool(name="qrp", bufs=6))   # qr rotation
            k_loc = dram.tile([KV_DIM, SC], bf16, tag="k_loc")
            # V staged per head-half so heads 0-7 can gather early
            v_loc = [dram.tile([SC, 520], bf16, tag=f"v_loc{h}", name=f"v_loc{h}")
                     for h in range(2)]
            k_gat = [dram.tile([4 * 128, SC], bf16, tag=f"kgat{j}",
                               name=f"kgat{j}") for j in range(8)]
            v_gat = [dram.tile([4 * SC, 520], bf16, tag=f"vgat{h}",
                               name=f"vgat{h}") for h in range(2)]

            attnout = [persist.tile([128, SC], bf16, tag=f"ao{m}", name=f"ao{m}")
                       for m in range(16)]
            # Vfull[set][kc]: kt-chunk kc of heads 8*set..8*set+7 (65 cols/head)
            Vfull = [[vfp.tile([128, 520], bf16, tag=f"Vf{h}_{i}",
                               name=f"Vf{h}_{i}") for i in range(16)]
                     for h in range(2)]
            Kfull = {}
            qr = {}

            def fetch_wo_set(nb):
                tiles = [wop.tile([128, 512], bf16, tag=f"wo{mm}",
                                  name=f"wo{nb}_{mm}") for mm in range(16)]
                for mm in range(16):
                    nc.sync.dma_start(
                        out=tiles[mm][:],
                        in_=Wo_in[128 * mm:128 * (mm + 1),
                                  512 * nb:512 * (nb + 1)])
                return tiles

            wo_sets = {}

            with ExitStack() as istack:
                xtp = istack.enter_context(tc.tile_pool(name="xtp", bufs=1))
                wkp = istack.enter_context(tc.tile_pool(name="wkp", bufs=2))
                wqp = istack.enter_context(tc.tile_pool(name="wqp", bufs=2))
                rpool = istack.enter_context(tc.tile_pool(name="rpool", bufs=2))
                rtp = istack.enter_context(tc.tile_pool(name="rtp", bufs=4))
                ppool = istack.enter_context(tc.tile_pool(name="ppool", bufs=2, space="PSUM"))
                spool = istack.enter_context(tc.tile_pool(name="spool", bufs=1, space="PSUM"))
                pvpool = istack.enter_context(tc.tile_pool(name="pvpool", bufs=2, space="PSUM"))
                vastack = ExitStack()
                vapool = vastack.enter_context(tc.tile_pool(name="vapool", bufs=1))
                # ---------- input / weight streams ----------
                xT_all = xtp.tile([128, 16 * SC], bf16, tag="xT", name="xT")
                for h in range(4):
                    eng = nc.sync if h % 2 == 0 else nc.scalar
                    sl = slice(4 * SC * h, 4 * SC * (h + 1))
                    eng.dma_start(out=xT_all[:, sl], in_=xT_in[:, sl])
                def xTc(i, lo=0, hi=SC):
                    return xT_all[:, SC * i + lo:SC * i + hi]

                wkc = {}
                def fetch_wk(j, eng):
                    w = wkp.tile([128, 2048], bf16, tag="wkc", name=f"wkc{j}")
                    eng.dma_start(out=w[:], in_=Wk_in[j])
                    wkc[j] = w

                wqc = {}
                def fetch_wq(m, eng):
                    w = wqp.tile([128, 2048], bf16, tag="wqc", name=f"wqc{m}")
                    eng.dma_start(out=w[:], in_=Wq_in[m])
                    wqc[m] = w

                fetch_wk(0, nc.scalar)
                fetch_wk(1, nc.scalar)

                # prime the Exp table so the first stream activation is fast
                dummy = rpool.tile([1, 64], bf16, tag="dummy")
                nc.vector.memset(dummy[:], 0.0)
                nc.scalar.activation(dummy[:], dummy[:], Exp)

                fetch_wq(0, nc.sync)
                fetch_wq(1, nc.sync)

                rts = {}
                def fetch_rt(chunk, eng):
                    rt = rtp.tile([128, 2 * SC], bf16, tag="rt", name=f"rt{chunk}")
                    eng.dma_start(out=rt[:], in_=rt_in[chunk])
                    rts[chunk] = rt

                fetch_rt(0, nc.sync)
                fetch_rt(1, nc.sync)
                fetch_rt(8, nc.sync)
                fetch_rt(9, nc.sync)

                # V-proj weights early on the scalar queue (pure DMAs first,
                # so later dep-waiting ops never block the queue head).
                # Weight tiles borrow the Wo pool (dead until block 11).
                wv_sets = []
                for nb in range(2):
                    wv = [wop.tile([128, 512], bf16, tag=f"wo{kc}",
                                   name=f"wv{nb}_{kc}") for kc in range(16)]
                    for kc in range(16):
                        nc.scalar.dma_start(
                            out=wv[kc][:],
                            in_=Wv_in[128 * kc:128 * (kc + 1),
                                      512 * nb:512 * (nb + 1)])
                    wv_sets.append(wv)

                psw = rpool.tile([128, 128], bf16, tag="psw", bufs=1)
                nc.sync.dma_start(out=psw[:], in_=psw_in[:])

                def rope(src_psum, dst_bf16, tab_chunk):
                    """dst = src*cos + swap(src)*signsin.  swap exchanges the
                    32-row evens/odds blocks within each 64-row head-half,
                    done by an in-place permutation matmul on the same PSUM
                    tile (qb and t1 are extracted first, so no extra bank)."""
                    rt = rts.pop(tab_chunk)
                    qb = rpool.tile([128, SC], bf16, tag="rope_qb", name="qb")
                    nc.vector.tensor_copy(qb[:], src_psum[:])
                    t1 = rpool.tile([128, SC], f32, tag="rope_t1", name="t1")
                    nc.vector.tensor_tensor(t1[:], src_psum[:], rt[:, 0:SC], mult)
                    nc.tensor.matmul(src_psum[:], psw[:], qb[:],
                                     start=True, stop=True)
                    t2 = rpool.tile([128, SC], bf16, tag="rope_t2", name="t2")
                    nc.vector.tensor_tensor(t2[:], src_psum[:], rt[:, SC:2 * SC], mult)
                    nc.vector.tensor_tensor(dst_bf16[:], t1[:], t2[:], add)

                # ---------- projection emitters ----------
                kfil_ps = {}

                def kproj_mm(j, kc):
                    if kc == 0:
                        kfil_ps[j] = ppool.tile([128, SC], f32, tag="proj",
                                                name=f"psk{j}")
                    nc.tensor.matmul(kfil_ps[j][:], wkc[j][:, 128 * kc:128 * (kc + 1)],
                                     xTc(kc), start=(kc == 0), stop=(kc == 15))

                def kproj_finish(j):
                    del wkc[j]
                    kr = rpool.tile([128, SC], bf16, tag="kr", name=f"kr{j}")
                    rope(kfil_ps.pop(j), kr, j)
                    nc.gpsimd.dma_start(out=k_loc[128 * j:128 * (j + 1), :],
                                        in_=kr[:])

                def kgather(j):
                    Kfull[j] = kfp.tile([128, 4 * SC], bf16, tag="Kf",
                                        name=f"Kf{j}")
                    nc.gpsimd.collective_compute(
                        "AllGather", mybir.AluOpType.bypass,
                        replica_groups=GROUPS,
                        ins=[k_loc[128 * j:128 * (j + 1), :]],
                        outs=[k_gat[j][:]])
                    for c in range(4):
                        nc.gpsimd.dma_start(
                            out=Kfull[j][:, SC * c:SC * (c + 1)],
                            in_=k_gat[j][128 * c:128 * (c + 1), :])

                qproj_ps = {}

                def qproj_mm(m, kc):
                    if kc == 0:
                        qproj_ps[m] = ppool.tile([128, SC], f32, tag="proj",
                                                 name=f"psq{m}")
                    nc.tensor.matmul(qproj_ps[m][:],
                                     wqc[m][:, 128 * kc:128 * (kc + 1)],
                                     xTc(kc), start=(kc == 0), stop=(kc == 15))

                def qproj_finish(m):
                    qr[m] = qrp.tile([128, SC], bf16, tag="qr", name=f"qr{m}")
                    rope(qproj_ps.pop(m), qr[m], 8 + m)
                    del wqc[m]

                # ---------- lead-in ----------
                va = [vapool.tile([128, 1040], bf16, tag=f"vaug{t}", name=f"vaug{t}")
                      for t in range(4)]
                for t in range(4):
                    nc.vector.memset(va[t][:], 1.0)

                def vproj_half(hh):
                    # V projection for heads 8*hh..8*hh+7 (ones col per head)
                    wv = wv_sets[hh]
                    for t in range(4):
                        ps = ppool.tile([128, SC], f32, tag="proj", name="psv")
                        for kc in range(16):
                            nc.tensor.matmul(
                                ps[:], xTc(kc, 128 * t, 128 * (t + 1)),
                                wv[kc][:],
                                start=(kc == 0), stop=(kc == 15))
                        dst = va[t][:, 520 * hh:520 * (hh + 1)].rearrange(
                            "p (h d) -> p h d", h=8)[:, :, 0:64]
                        src = ps[:].rearrange("p (h d) -> p h d", h=8)
                        nc.vector.tensor_copy(dst, src)
                    for t in range(4):
                        nc.scalar.dma_start(
                            out=v_loc[hh][128 * t:128 * (t + 1), :],
                            in_=va[t][:, 520 * hh:520 * (hh + 1)])

                def vgather(hh):
                    nc.gpsimd.collective_compute(
                        "AllGather", mybir.AluOpType.bypass, replica_groups=GROUPS,
                        ins=[v_loc[hh][:]], outs=[v_gat[hh][:]])
                    for i in range(16):
                        nc.gpsimd.dma_start(
                            out=Vfull[hh][i][:],
                            in_=v_gat[hh][128 * i:128 * (i + 1), :])

                # PE order interleaves K/V/Q so the CC stream (g0 | vA | g1 | vB)
                # gets its inputs early while qr0 also lands early
                for kc in range(16):
                    kproj_mm(0, kc)
                kproj_finish(0)
                kgather(0)
                vproj_half(0)
                vgather(0)
                fetch_wq(2, nc.sync)
                for kc in range(16):
                    qproj_mm(0, kc)
                qproj_finish(0)
                for kc in range(16):
                    kproj_mm(1, kc)
                kproj_finish(1)
                kgather(1)
                vproj_half(1)
                vgather(1)
                for kc in range(16):
                    qproj_mm(1, kc)
                qproj_finish(1)
                # va/wv space is dead from here; let epool/npool reuse it
                vastack.close()
                epool = istack.enter_context(tc.tile_pool(name="epool", bufs=14))
                npool = istack.enter_context(tc.tile_pool(name="npool", bufs=2))

                # ---------- attention stream ----------
                e_tiles = {}

                def emit_scores(m, kp):
                    j = m // 2
                    spA = spool.tile([128, 1024], f32, tag="spA", name="spA")
                    spB = spool.tile([128, 1024], f32, tag="spB", name="spB")
                    sp = [spB, spA]  # half0 -> B, half1 -> A
                    # half-major, h1 first: h1's scores finish before the PE
                    # queue blocks on h0's later WAR release, so exp(s+1,h1)
                    # is never delayed through h0's wait
                    for half in (1, 0):
                        for u in range(2):
                            kc = 2 * kp + u
                            nc.tensor.matmul(
                                sp[half][:, 512 * u:512 * (u + 1)],
                                Kfull[j][64 * half:64 * (half + 1),
                                         128 * kc:128 * (kc + 1)],
                                qr[m][64 * half:64 * (half + 1), :],
                                start=True, stop=True)
                    e = [None, None]
                    for half in (1, 0):
                        et = epool.tile([128, 1024], bf16, tag="exp", name="et")
                        nc.scalar.activation(et[:], sp[half][:], Exp, scale=0.125)
                        e[half] = et
                    e_tiles[(m, kp)] = e

                pv_of = {}

                pending_norms = []

                def emit_norm(m, pv):
                    # part 1: drain pv + reciprocal; the gpsimd broadcast and
                    # final mult are deferred a few steps so a broadcast stuck
                    # behind a collective never blocks the DVE queue head
                    recs, raws = [], []
                    for half in range(2):
                        dex = npool.tile([1, SC], f32, tag="dex", bufs=2, name="dex")
                        nc.vector.tensor_copy(dex[:], pv[half][64:65, :])
                        raw = npool.tile([65, SC], bf16, tag="raw", bufs=4, name="raw")
                        nc.vector.tensor_copy(raw[:], pv[half][:])
                        rec = npool.tile([1, SC], f32, tag="rec", bufs=2, name="rec")
                        nc.vector.reciprocal_approx_fast(out=rec[:], in_=dex[:])
                        bcs = npool.tile([64, SC], f32, tag="bcs", bufs=2, name="bcs")
                        nc.gpsimd.partition_broadcast(bcs[:], rec[:], channels=64)
                        recs.append(bcs)
                        raws.append(raw)
                    pending_norms.append((m, recs, raws))

                def norm_part2():
                    m, bcss, raws = pending_norms.pop(0)
                    for half in range(2):
                        nc.vector.tensor_tensor(
                            attnout[m][64 * half:64 * (half + 1), :],
                            raws[half][0:64, :], bcss[half][:], mult)

                def emit_pv(s2):
                    pm, kp = s2 // 8, s2 % 8
                    if kp == 0:
                        pv_of[pm] = [pvpool.tile([65, SC], f32, tag="pv",
                                                 name="pv") for _ in range(2)]
                    pv = pv_of[pm]
                    e = e_tiles.pop((pm, kp))
                    j = pm // 2
                    for half in range(2):
                        g = 2 * j + half
                        hs, gc = g // 8, g % 8
                        for u in range(2):
                            kc = 2 * kp + u
                            nc.tensor.matmul(
                                pv[half][:],
                                Vfull[hs][kc][:, 65 * gc:65 * (gc + 1)],
                                e[half][:, 512 * u:512 * (u + 1)],
                                start=(kp == 0 and u == 0),
                                stop=(kp == 7 and u == 1))
                    if kp == 7:
                        emit_norm(pm, pv_of.pop(pm))

                for s in range(128):
                    m, kp = s // 8, s % 8
                    # PV (lagged)
                    if s >= LAG:
                        emit_pv(s - LAG)
                    if pending_norms and s >= 8 * pending_norms[0][0] + 13 + 6:
                        norm_part2()
                    # K-proj filler in blocks 0-5 (j = 2..7)
                    if m < 6:
                        jf = 2 + m
                        if kp == 0:
                            fetch_wk(jf, nc.sync)
                            fetch_rt(jf, nc.sync)
                        for kc in (2 * kp, 2 * kp + 1):
                            kproj_mm(jf, kc)
                        if kp == 7:
                            kproj_finish(jf)
                            kgather(jf)
                    # Q-proj filler (m+2), 2 matmuls per step
                    mq = m + 2
                    if mq < 16:
                        if kp == 0:
                            if mq + 1 < 16:
                                fetch_wq(mq + 1, nc.sync)
                            fetch_rt(8 + mq, nc.sync)
                        for kc in (2 * kp, 2 * kp + 1):
                            qproj_mm(mq, kc)
                        if kp == 7:
                            qproj_finish(mq)
                    # Wo prefetch waves late in the stream
                    if m == 11 and kp == 0:
                        wo_sets[0] = fetch_wo_set(0)
                    if m == 13 and kp == 0:
                        wo_sets[1] = fetch_wo_set(1)
                    # scores + exp for this step
                    emit_scores(m, kp)

                # drain remaining PV groups + norms
                for s2 in range(128 - LAG, 128):
                    emit_pv(s2)
                while pending_norms:
                    norm_part2()

            # ---------- O projection tail ----------
            with ExitStack() as tstack:
                opsum = tstack.enter_context(tc.tile_pool(name="opsum", bufs=2, space="PSUM"))
                ostage = tstack.enter_context(tc.tile_pool(name="ostage", bufs=4))
                wo_sets[2] = fetch_wo_set(2)
                for nb in range(4):
                    if nb == 1:
                        wo_sets[3] = fetch_wo_set(3)
                    wset = wo_sets[nb]
                    for t in range(4):
                        ps = opsum.tile([128, 512], f32, tag="ops")
                        for mm in range(16):
                            nc.tensor.matmul(
                                ps[:],
                                attnout[mm][:, 128 * t:128 * (t + 1)],
                                wset[mm][:],
                                start=(mm == 0), stop=(mm == 15))
                        ot = ostage.tile([128, 512], f32, tag="ot")
                        nc.scalar.copy(ot[:], ps[:])
                        eng = nc.gpsimd if (t % 2 == 0) else nc.scalar
                        eng.dma_start(
                            out=out_dram[128 * t:128 * (t + 1),
                                         512 * nb:512 * (nb + 1)],
                            in_=ot[:])

    nc.compile()
    _cache["nc"] = nc
    return nc


def kernel(x, Wq, Wk, Wv, Wo):
    from concourse.bass_utils import run_bass_kernel_spmd

    _host_prep()
    x = np.asarray(x, dtype=np.float32)
    qperm = _cache["qperm"]
    kperm = _cache["kperm"]
    perm = _cache["perm"]
    Wq_perm = np.asarray(Wq, dtype=np.float32)[:, qperm].astype(BF16)
    Wk_perm = np.asarray(Wk, dtype=np.float32)[:, kperm].astype(BF16)
    # tile for contiguous per-chunk loads: W_t[chunk, p, kc*128+c] = W[128*kc+p, 128*chunk+c]
    Wq_t = np.ascontiguousarray(
        Wq_perm.reshape(16, 128, 16, 128).transpose(2, 1, 0, 3).reshape(16, 128, D_MODEL))
    Wk_t = np.ascontiguousarray(
        Wk_perm.reshape(16, 128, 8, 128).transpose(2, 1, 0, 3).reshape(8, 128, D_MODEL))
    Wv_b = np.asarray(Wv, dtype=np.float32).astype(BF16)
    Wo_perm = np.ascontiguousarray(np.asarray(Wo, dtype=np.float32)[perm, :]).astype(BF16)

    in_maps = []
    for core in range(N_CORES):
        b, ci = core // 4, core % 4
        xT = np.ascontiguousarray(x[b, ci * SC:(ci + 1) * SC, :].T).astype(BF16)
        xT_t = np.ascontiguousarray(
            xT.reshape(16, 128, SC).transpose(1, 0, 2).reshape(128, 16 * SC))
        in_maps.append({
            "xT": xT_t, "Wq": Wq_t, "Wk": Wk_t, "Wv": Wv_b, "Wo": Wo_perm,
            "ropetab": _cache["tabs"][ci], "Pswap": _cache["Pswap"],
        })
    _cache["in_maps"] = in_maps

    nc = _build_nc()
    res = run_bass_kernel_spmd(nc, in_maps, list(range(N_CORES)))
    out = np.zeros((B, S, D_MODEL), dtype=np.float32)
    for core in range(N_CORES):
        b, ci = core // 4, core % 4
        out[b, ci * SC:(ci + 1) * SC, :] = res.results[core]["out"]
    return out


# revision 52
# speedup vs baseline: 1.0102x; 1.0102x over previous
"""GQA attention kernel for 8 TRN2 NeuronCores (Bass/Tile) — v3.

Sharding: tokens sharded 8 ways (2 batches x 4 chunks of 512).  Each core
computes Q/K/V projections for its 512 tokens in a transposed
(feature-on-partition) layout, all-gathers K/V within its 4-core batch
group, then runs attention with scores in [k_tok, q_tok] layout and a
ones column appended to V so the softmax denominator falls out of the PV
matmul.  Output token rows are disjoint per core -> no collective for O.

v3 schedule: one fused software-pipelined stream.  Lead-in computes
K proj j=0,1 (j-granular gathers so attention can start ~35us in),
V proj (+ split V gather), Q proj m=0,1.  Then a 128-step attention
stream (one step per (m, kp)) keeps the scalar engine (exp) saturated;
PE slack in each step is filled with the remaining K projections
(blocks 0-5) and just-in-time Q projections (m+2 per block).  PV lags
LAG steps behind scores/exp and consumes KV chunks in gather-arrival
order (even kp first).  O projection runs as a tail with Wo streamed
in per-nb waves.

RoPE pair-swap is 4 contiguous-partition SBUF->SBUF DMA copies instead
of a permutation matmul: Wq/Wk columns are host-permuted so each 64-row
head-half holds [evens | odds]; bf16 cos/sin tables (half the DMA bytes
of f32) are built to match.  Softmax denominators are inverted with the
fast fp32 reciprocal approximation (~5x faster than nc.vector.reciprocal).

All matmuls bf16 (fp32 is 4x slower on the PE); fp32 PSUM accum.
PSUM budget: proj 2 banks + scores 2x2 banks + PV 2 banks = 8.
"""
import numpy as np
import ml_dtypes

D_MODEL = 2048
KV_DIM = 1024
B = 2
S = 2048
SC = 512            # tokens per core
N_CORES = 8
ROPE_BASE = 10000.0
BF16 = ml_dtypes.bfloat16

_cache = {}


def _host_prep():
    if "perm" in _cache:
        return
    # head-pairing permutation (as v2): row chunk m pairs the two query
    # heads whose KV-head halves share a K chunk.
    perm = np.zeros(D_MODEL, dtype=np.int64)
    for g in range(16):
        for qi in range(2):
            for d in range(64):
                f = g * 128 + qi * 64 + d
                p = ((g // 2) * 2 + qi) * 128 + (g % 2) * 64 + d
                perm[p] = f
    _cache["perm"] = perm  # Wo rows keep this (attnout dims are not eo-split)

    # evens-then-odds split within each 64-row head-half, so the rope
    # pair-swap becomes two contiguous 32-partition block swaps.
    eo = np.concatenate([np.arange(0, 64, 2), np.arange(1, 64, 2)])
    qperm = np.zeros(D_MODEL, dtype=np.int64)
    for b64 in range(D_MODEL // 64):
        qperm[64 * b64:64 * (b64 + 1)] = perm[64 * b64 + eo]
    _cache["qperm"] = qperm
    kperm = np.zeros(KV_DIM, dtype=np.int64)
    for b64 in range(KV_DIM // 64):
        kperm[64 * b64:64 * (b64 + 1)] = 64 * b64 + eo
    _cache["kperm"] = kperm

    theta = ROPE_BASE ** (-np.arange(1024, dtype=np.float64) / 1024.0)
    tabs = []
    for ci in range(4):
        pos = np.arange(ci * SC, (ci + 1) * SC, dtype=np.float64)
        tab = np.zeros((24, 128, 2 * SC), dtype=np.float64)
        for c in range(24):
            if c < 8:
                flat = kperm[np.arange(128 * c, 128 * (c + 1))]
            else:
                flat = qperm[128 * (c - 8):128 * (c - 7)]
            ang = theta[flat // 2][:, None] * pos[None, :]
            sign = np.where(flat % 2 == 0, -1.0, 1.0)
            tab[c, :, :SC] = np.cos(ang)
            tab[c, :, SC:] = sign[:, None] * np.sin(ang)
        tabs.append(tab.astype(BF16))
    _cache["tabs"] = tabs

    # 32-row block-swap permutation (evens block <-> odds block per head-half)
    Pswap = np.zeros((128, 128), dtype=np.float32)
    for h in range(2):
        for b in range(32):
            Pswap[64 * h + 32 + b, 64 * h + b] = 1.0
            Pswap[64 * h + b, 64 * h + 32 + b] = 1.0
    _cache["Pswap"] = Pswap.astype(BF16)


def _build_nc():
    if "nc" in _cache:
        return _cache["nc"]
    import concourse.bacc as bacc
    import concourse.mybir as mybir
    import concourse.tile as tile

    f32 = mybir.dt.float32
    bf16 = mybir.dt.bfloat16
    Exp = mybir.ActivationFunctionType.Exp
    mult = mybir.AluOpType.mult
    add = mybir.AluOpType.add

    nc = bacc.Bacc("TRN2", target_bir_lowering=False, debug=False,
                   num_devices=N_CORES)

    # xT/Wq/Wk are host-tiled so every SBUF load is one contiguous
    # full-rate DMA ([128, 2048] per chunk; the strided/rearranged loads
    # measured ~82GB/s and clogged the queues)
    xT_in = nc.dram_tensor("xT", [128, 16 * SC], bf16, kind="ExternalInput").ap()
    Wq_in = nc.dram_tensor("Wq", [16, 128, D_MODEL], bf16, kind="ExternalInput").ap()
    Wk_in = nc.dram_tensor("Wk", [8, 128, D_MODEL], bf16, kind="ExternalInput").ap()
    Wv_in = nc.dram_tensor("Wv", [D_MODEL, KV_DIM], bf16, kind="ExternalInput").ap()
    Wo_in = nc.dram_tensor("Wo", [D_MODEL, D_MODEL], bf16, kind="ExternalInput").ap()
    rt_in = nc.dram_tensor("ropetab", [24, 128, 2 * SC], bf16,
                           kind="ExternalInput").ap()
    psw_in = nc.dram_tensor("Pswap", [128, 128], bf16, kind="ExternalInput").ap()
    out_dram = nc.dram_tensor("out", [SC, D_MODEL], f32, kind="ExternalOutput").ap()

    GROUPS = [[0, 1, 2, 3], [4, 5, 6, 7]]
    LAG = 6

    from contextlib import ExitStack

    with tile.TileContext(nc) as tc, nc.allow_low_precision(reason="bf16 matmul pipeline by design"):
        with ExitStack() as ostack:
            dram = ostack.enter_context(tc.tile_pool(name="dram", bufs=1, space="DRAM"))
            persist = ostack.enter_context(tc.tile_pool(name="persist", bufs=1))
            kfp = ostack.enter_context(tc.tile_pool(name="kfp", bufs=5))   # Kfull rotation
            vfp = ostack.enter_context(tc.tile_pool(name="vfp", bufs=1))   # Vfull (static)
            wop = ostack.enter_context(tc.tile_pool(name="wop", bufs=2))   # Wv halves + Wo waves
            qrp = ostack.enter_context(tc.tile_pool(name="qrp", bufs=6))   # qr rotation
            k_loc = dram.tile([KV_DIM, SC], bf16, tag="k_loc")
            # V staged per head-half so heads 0-7 can gather early
            v_loc = [dram.tile([SC, 520], bf16, tag=f"v_loc{h}", name=f"v_loc{h}")
                     for h in range(2)]
            k_gat = [dram.tile([4 * 128, SC], bf16, tag=f"kgat{j}",
                               name=f"kgat{j}") for j in range(2)]
            k_gat2 = dram.tile([2 * 4 * 128, SC], bf16, tag="kgat23")
            k_gat4 = dram.tile([4 * 4 * 128, SC], bf16, tag="kgat47")
            v_gat = [dram.tile([4 * SC, 520], bf16, tag=f"vgat{h}",
                               name=f"vgat{h}") for h in range(2)]

            attnout = [persist.tile([128, SC], bf16, tag=f"ao{m}", name=f"ao{m}")
                       for m in range(16)]
            # Vfull[set][kc]: kt-chunk kc of heads 8*set..8*set+7 (65 cols/head)
            Vfull = [[vfp.tile([128, 520], bf16, tag=f"Vf{h}_{i}",
                               name=f"Vf{h}_{i}") for i in range(16)]
                     for h in range(2)]
            Kfull = {}
            qr = {}

            def fetch_wo_set(nb):
                tiles = [wop.tile([128, 512], bf16, tag=f"wo{mm}",
                                  name=f"wo{nb}_{mm}") for mm in range(16)]
                for mm in range(16):
                    nc.sync.dma_start(
                        out=tiles[mm][:],
                        in_=Wo_in[128 * mm:128 * (mm + 1),
                                  512 * nb:512 * (nb + 1)])
                return tiles

            wo_sets = {}

            with ExitStack() as istack:
                xtp = istack.enter_context(tc.tile_pool(name="xtp", bufs=1))
                wkp = istack.enter_context(tc.tile_pool(name="wkp", bufs=2))
                wqp = istack.enter_context(tc.tile_pool(name="wqp", bufs=2))
                rpool = istack.enter_context(tc.tile_pool(name="rpool", bufs=2))
                rtp = istack.enter_context(tc.tile_pool(name="rtp", bufs=4))
                ppool = istack.enter_context(tc.tile_pool(name="ppool", bufs=2, space="PSUM"))
                spool = istack.enter_context(tc.tile_pool(name="spool", bufs=1, space="PSUM"))
                pvpool = istack.enter_context(tc.tile_pool(name="pvpool", bufs=2, space="PSUM"))
                vastack = ExitStack()
                vapool = vastack.enter_context(tc.tile_pool(name="vapool", bufs=1))
                # ---------- input / weight streams ----------
                xT_all = xtp.tile([128, 16 * SC], bf16, tag="xT", name="xT")
                for h in range(4):
                    eng = nc.sync if h % 2 == 0 else nc.scalar
                    sl = slice(4 * SC * h, 4 * SC * (h + 1))
                    eng.dma_start(out=xT_all[:, sl], in_=xT_in[:, sl])
                def xTc(i, lo=0, hi=SC):
                    return xT_all[:, SC * i + lo:SC * i + hi]

                wkc = {}
                def fetch_wk(j, eng):
                    w = wkp.tile([128, 2048], bf16, tag="wkc", name=f"wkc{j}")
                    eng.dma_start(out=w[:], in_=Wk_in[j])
                    wkc[j] = w

                wqc = {}
                def fetch_wq(m, eng):
                    w = wqp.tile([128, 2048], bf16, tag="wqc", name=f"wqc{m}")
                    eng.dma_start(out=w[:], in_=Wq_in[m])
                    wqc[m] = w

                fetch_wk(0, nc.scalar)
                fetch_wk(1, nc.scalar)

                # prime the Exp table so the first stream activation is fast
                dummy = rpool.tile([1, 64], bf16, tag="dummy")
                nc.vector.memset(dummy[:], 0.0)
                nc.scalar.activation(dummy[:], dummy[:], Exp)

                fetch_wq(0, nc.sync)
                fetch_wq(1, nc.sync)

                rts = {}
                def fetch_rt(chunk, eng):
                    rt = rtp.tile([128, 2 * SC], bf16, tag="rt", name=f"rt{chunk}")
                    eng.dma_start(out=rt[:], in_=rt_in[chunk])
                    rts[chunk] = rt

                fetch_rt(0, nc.sync)
                fetch_rt(1, nc.sync)
                fetch_rt(8, nc.sync)
                fetch_rt(9, nc.sync)

                # V-proj weights early on the scalar queue (pure DMAs first,
                # so later dep-waiting ops never block the queue head).
                # Weight tiles borrow the Wo pool (dead until block 11).
                wv_sets = []
                for nb in range(2):
                    wv = [wop.tile([128, 512], bf16, tag=f"wo{kc}",
                                   name=f"wv{nb}_{kc}") for kc in range(16)]
                    for kc in range(16):
                        nc.scalar.dma_start(
                            out=wv[kc][:],
                            in_=Wv_in[128 * kc:128 * (kc + 1),
                                      512 * nb:512 * (nb + 1)])
                    wv_sets.append(wv)

                psw = rpool.tile([128, 128], bf16, tag="psw", bufs=1)
                nc.sync.dma_start(out=psw[:], in_=psw_in[:])

                def rope(src_psum, dst_bf16, tab_chunk):
                    """dst = src*cos + swap(src)*signsin.  swap exchanges the
                    32-row evens/odds blocks within each 64-row head-half,
                    done by an in-place permutation matmul on the same PSUM
                    tile (qb and t1 are extracted first, so no extra bank)."""
                    rt = rts.pop(tab_chunk)
                    qb = rpool.tile([128, SC], bf16, tag="rope_qb", name="qb")
                    nc.vector.tensor_copy(qb[:], src_psum[:])
                    t1 = rpool.tile([128, SC], f32, tag="rope_t1", name="t1")
                    nc.vector.tensor_tensor(t1[:], src_psum[:], rt[:, 0:SC], mult)
                    nc.tensor.matmul(src_psum[:], psw[:], qb[:],
                                     start=True, stop=True)
                    t2 = rpool.tile([128, SC], bf16, tag="rope_t2", name="t2")
                    nc.vector.tensor_tensor(t2[:], src_psum[:], rt[:, SC:2 * SC], mult)
                    nc.vector.tensor_tensor(dst_bf16[:], t1[:], t2[:], add)

                # ---------- projection emitters ----------
                kfil_ps = {}

                def kproj_mm(j, kc):
                    if kc == 0:
                        kfil_ps[j] = ppool.tile([128, SC], f32, tag="proj",
                                                name=f"psk{j}")
                    nc.tensor.matmul(kfil_ps[j][:], wkc[j][:, 128 * kc:128 * (kc + 1)],
                                     xTc(kc), start=(kc == 0), stop=(kc == 15))

                def kproj_finish(j):
                    del wkc[j]
                    kr = rpool.tile([128, SC], bf16, tag="kr", name=f"kr{j}")
                    rope(kfil_ps.pop(j), kr, j)
                    nc.gpsimd.dma_start(out=k_loc[128 * j:128 * (j + 1), :],
                                        in_=kr[:])

                def kgather(j):
                    Kfull[j] = kfp.tile([128, 4 * SC], bf16, tag="Kf",
                                        name=f"Kf{j}")
                    nc.gpsimd.collective_compute(
                        "AllGather", mybir.AluOpType.bypass,
                        replica_groups=GROUPS,
                        ins=[k_loc[128 * j:128 * (j + 1), :]],
                        outs=[k_gat[j][:]])
                    for c in range(4):
                        nc.gpsimd.dma_start(
                            out=Kfull[j][:, SC * c:SC * (c + 1)],
                            in_=k_gat[j][128 * c:128 * (c + 1), :])

                def kgather_group(j0, nj, gat):
                    # one collective for K chunks j0..j0+nj-1 (fewer, bigger
                    # CC ops: less per-op overhead and less gpsimd blocking)
                    for j in range(j0, j0 + nj):
                        Kfull[j] = kfp.tile([128, 4 * SC], bf16, tag="Kf",
                                            name=f"Kf{j}")
                    nc.gpsimd.collective_compute(
                        "AllGather", mybir.AluOpType.bypass,
                        replica_groups=GROUPS,
                        ins=[k_loc[128 * j0:128 * (j0 + nj), :]],
                        outs=[gat[:]])
                    for j in range(j0, j0 + nj):
                        for c in range(4):
                            nc.gpsimd.dma_start(
                                out=Kfull[j][:, SC * c:SC * (c + 1)],
                                in_=gat[128 * (nj * c + (j - j0)):
                                        128 * (nj * c + (j - j0)) + 128, :])

                qproj_ps = {}

                def qproj_mm(m, kc):
                    if kc == 0:
                        qproj_ps[m] = ppool.tile([128, SC], f32, tag="proj",
                                                 name=f"psq{m}")
                    nc.tensor.matmul(qproj_ps[m][:],
                                     wqc[m][:, 128 * kc:128 * (kc + 1)],
                                     xTc(kc), start=(kc == 0), stop=(kc == 15))

                def qproj_finish(m):
                    qr[m] = qrp.tile([128, SC], bf16, tag="qr", name=f"qr{m}")
                    rope(qproj_ps.pop(m), qr[m], 8 + m)
                    del wqc[m]

                # ---------- lead-in ----------
                va = [vapool.tile([128, 1040], bf16, tag=f"vaug{t}", name=f"vaug{t}")
                      for t in range(4)]
                for t in range(4):
                    nc.vector.memset(va[t][:], 1.0)

                def vproj_half(hh):
                    # V projection for heads 8*hh..8*hh+7 (ones col per head)
                    wv = wv_sets[hh]
                    for t in range(4):
                        ps = ppool.tile([128, SC], f32, tag="proj", name="psv")
                        for kc in range(16):
                            nc.tensor.matmul(
                                ps[:], xTc(kc, 128 * t, 128 * (t + 1)),
                                wv[kc][:],
                                start=(kc == 0), stop=(kc == 15))
                        dst = va[t][:, 520 * hh:520 * (hh + 1)].rearrange(
                            "p (h d) -> p h d", h=8)[:, :, 0:64]
                        src = ps[:].rearrange("p (h d) -> p h d", h=8)
                        nc.vector.tensor_copy(dst, src)
                    for t in range(4):
                        nc.scalar.dma_start(
                            out=v_loc[hh][128 * t:128 * (t + 1), :],
                            in_=va[t][:, 520 * hh:520 * (hh + 1)])

                def vgather(hh):
                    nc.gpsimd.collective_compute(
                        "AllGather", mybir.AluOpType.bypass, replica_groups=GROUPS,
                        ins=[v_loc[hh][:]], outs=[v_gat[hh][:]])
                    for i in range(16):
                        nc.gpsimd.dma_start(
                            out=Vfull[hh][i][:],
                            in_=v_gat[hh][128 * i:128 * (i + 1), :])

                # PE order interleaves K/V/Q so the CC stream (g0 | vA | g1 | vB)
                # gets its inputs early while qr0 also lands early
                for kc in range(16):
                    kproj_mm(0, kc)
                kproj_finish(0)
                kgather(0)
                vproj_half(0)
                vgather(0)
                fetch_wq(2, nc.sync)
                for kc in range(16):
                    qproj_mm(0, kc)
                qproj_finish(0)
                for kc in range(16):
                    kproj_mm(1, kc)
                kproj_finish(1)
                kgather(1)
                vproj_half(1)
                vgather(1)
                for kc in range(16):
                    qproj_mm(1, kc)
                qproj_finish(1)
                # va/wv space is dead from here; let epool/npool reuse it
                vastack.close()
                epool = istack.enter_context(tc.tile_pool(name="epool", bufs=14))
                npool = istack.enter_context(tc.tile_pool(name="npool", bufs=2))

                # ---------- attention stream ----------
                e_tiles = {}

                def emit_scores(m, kp):
                    j = m // 2
                    spA = spool.tile([128, 1024], f32, tag="spA", name="spA")
                    spB = spool.tile([128, 1024], f32, tag="spB", name="spB")
                    sp = [spB, spA]  # half0 -> B, half1 -> A
                    for u in range(2):
                        for half in (1, 0):
                            kc = 2 * kp + u
                            nc.tensor.matmul(
                                sp[half][:, 512 * u:512 * (u + 1)],
                                Kfull[j][64 * half:64 * (half + 1),
                                         128 * kc:128 * (kc + 1)],
                                qr[m][64 * half:64 * (half + 1), :],
                                start=True, stop=True)
                    e = [None, None]
                    for half in (1, 0):
                        et = epool.tile([128, 1024], bf16, tag="exp", name="et")
                        nc.scalar.activation(et[:], sp[half][:], Exp, scale=0.125)
                        e[half] = et
                    e_tiles[(m, kp)] = e

                pv_of = {}

                pending_norms = []

                def emit_norm(m, pv):
                    # part 1: drain pv + reciprocal; the gpsimd broadcast and
                    # final mult are deferred a few steps so a broadcast stuck
                    # behind a collective never blocks the DVE queue head
                    recs, raws = [], []
                    for half in range(2):
                        dex = npool.tile([1, SC], f32, tag="dex", bufs=2, name="dex")
                        nc.vector.tensor_copy(dex[:], pv[half][64:65, :])
                        raw = npool.tile([65, SC], bf16, tag="raw", bufs=4, name="raw")
                        nc.vector.tensor_copy(raw[:], pv[half][:])
                        rec = npool.tile([1, SC], f32, tag="rec", bufs=2, name="rec")
                        nc.vector.reciprocal_approx_fast(out=rec[:], in_=dex[:])
                        bcs = npool.tile([64, SC], f32, tag="bcs", bufs=2, name="bcs")
                        nc.gpsimd.partition_broadcast(bcs[:], rec[:], channels=64)
                        recs.append(bcs)
                        raws.append(raw)
                    pending_norms.append((m, recs, raws))

                def norm_part2():
                    m, bcss, raws = pending_norms.pop(0)
                    for half in range(2):
                        nc.vector.tensor_tensor(
                            attnout[m][64 * half:64 * (half + 1), :],
                            raws[half][0:64, :], bcss[half][:], mult)

                def emit_pv(s2):
                    pm, kp = s2 // 8, s2 % 8
                    if kp == 0:
                        pv_of[pm] = [pvpool.tile([65, SC], f32, tag="pv",
                                                 name="pv") for _ in range(2)]
                    pv = pv_of[pm]
                    e = e_tiles.pop((pm, kp))
                    j = pm // 2
                    for half in range(2):
                        g = 2 * j + half
                        hs, gc = g // 8, g % 8
                        for u in range(2):
                            kc = 2 * kp + u
                            nc.tensor.matmul(
                                pv[half][:],
                                Vfull[hs][kc][:, 65 * gc:65 * (gc + 1)],
                                e[half][:, 512 * u:512 * (u + 1)],
                                start=(kp == 0 and u == 0),
                                stop=(kp == 7 and u == 1))
                    if kp == 7:
                        emit_norm(pm, pv_of.pop(pm))

                for s in range(128):
                    m, kp = s // 8, s % 8
                    # PV (lagged)
                    if s >= LAG:
                        emit_pv(s - LAG)
                    if pending_norms and s >= 8 * pending_norms[0][0] + 13 + 6:
                        norm_part2()
                    # K-proj filler in blocks 0-5 (j = 2..7)
                    if m < 6:
                        jf = 2 + m
                        if kp == 0:
                            fetch_wk(jf, nc.sync)
                            fetch_rt(jf, nc.sync)
                        for kc in (2 * kp, 2 * kp + 1):
                            kproj_mm(jf, kc)
                        if kp == 7:
                            kproj_finish(jf)
                            if jf == 3:
                                kgather_group(2, 2, k_gat2)
                            elif jf == 7:
                                kgather_group(4, 4, k_gat4)
                    # Q-proj filler (m+2), 2 matmuls per step
                    mq = m + 2
                    if mq < 16:
                        if kp == 0:
                            if mq + 1 < 16:
                                fetch_wq(mq + 1, nc.sync)
                            fetch_rt(8 + mq, nc.sync)
                        for kc in (2 * kp, 2 * kp + 1):
                            qproj_mm(mq, kc)
                        if kp == 7:
                            qproj_finish(mq)
                    # Wo prefetch waves late in the stream
                    if m == 11 and kp == 0:
                        wo_sets[0] = fetch_wo_set(0)
                    if m == 13 and kp == 0:
                        wo_sets[1] = fetch_wo_set(1)
                    # scores + exp for this step
                    emit_scores(m, kp)

                # drain remaining PV groups + norms
                for s2 in range(128 - LAG, 128):
                    emit_pv(s2)
                while pending_norms:
                    norm_part2()

            # ---------- O projection tail ----------
            with ExitStack() as tstack:
                opsum = tstack.enter_context(tc.tile_pool(name="opsum", bufs=2, space="PSUM"))
                ostage = tstack.enter_context(tc.tile_pool(name="ostage", bufs=4))
                wo_sets[2] = fetch_wo_set(2)
                for nb in range(4):
                    if nb == 1:
                        wo_sets[3] = fetch_wo_set(3)
                    wset = wo_sets[nb]
                    for t in range(4):
                        ps = opsum.tile([128, 512], f32, tag="ops")
                        for mm in range(16):
                            nc.tensor.matmul(
                                ps[:],
                                attnout[mm][:, 128 * t:128 * (t + 1)],
                                wset[mm][:],
                                start=(mm == 0), stop=(mm == 15))
                        ot = ostage.tile([128, 512], f32, tag="ot")
                        nc.scalar.copy(ot[:], ps[:])
                        eng = nc.gpsimd if (t % 2 == 0) else nc.scalar
                        eng.dma_start(
                            out=out_dram[128 * t:128 * (t + 1),
                                         512 * nb:512 * (nb + 1)],
                            in_=ot[:])

    nc.compile()
    _cache["nc"] = nc
    return nc


def kernel(x, Wq, Wk, Wv, Wo):
    from concourse.bass_utils import run_bass_kernel_spmd

    _host_prep()
    x = np.asarray(x, dtype=np.float32)
    qperm = _cache["qperm"]
    kperm = _cache["kperm"]
    perm = _cache["perm"]
    Wq_perm = np.asarray(Wq, dtype=np.float32)[:, qperm].astype(BF16)
    Wk_perm = np.asarray(Wk, dtype=np.float32)[:, kperm].astype(BF16)
    # tile for contiguous per-chunk loads: W_t[chunk, p, kc*128+c] = W[128*kc+p, 128*chunk+c]
    Wq_t = np.ascontiguousarray(
        Wq_perm.reshape(16, 128, 16, 128).transpose(2, 1, 0, 3).reshape(16, 128, D_MODEL))
    Wk_t = np.ascontiguousarray(
        Wk_perm.reshape(16, 128, 8, 128).transpose(2, 1, 0, 3).reshape(8, 128, D_MODEL))
    Wv_b = np.asarray(Wv, dtype=np.float32).astype(BF16)
    Wo_perm = np.ascontiguousarray(np.asarray(Wo, dtype=np.float32)[perm, :]).astype(BF16)

    in_maps = []
    for core in range(N_CORES):
        b, ci = core // 4, core % 4
        xT = np.ascontiguousarray(x[b, ci * SC:(ci + 1) * SC, :].T).astype(BF16)
        xT_t = np.ascontiguousarray(
            xT.reshape(16, 128, SC).transpose(1, 0, 2).reshape(128, 16 * SC))
        in_maps.append({
            "xT": xT_t, "Wq": Wq_t, "Wk": Wk_t, "Wv": Wv_b, "Wo": Wo_perm,
            "ropetab": _cache["tabs"][ci], "Pswap": _cache["Pswap"],
        })
    _cache["in_maps"] = in_maps

    nc = _build_nc()
    res = run_bass_kernel_spmd(nc, in_maps, list(range(N_CORES)))
    out = np.zeros((B, S, D_MODEL), dtype=np.float32)
    for core in range(N_CORES):
        b, ci = core // 4, core % 4
        out[b, ci * SC:(ci + 1) * SC, :] = res.results[core]["out"]
    return out


# revision 53
# speedup vs baseline: 1.0228x; 1.0125x over previous
"""GQA attention kernel for 8 TRN2 NeuronCores (Bass/Tile) — v3.

Sharding: tokens sharded 8 ways (2 batches x 4 chunks of 512).  Each core
computes Q/K/V projections for its 512 tokens in a transposed
(feature-on-partition) layout, all-gathers K/V within its 4-core batch
group, then runs attention with scores in [k_tok, q_tok] layout and a
ones column appended to V so the softmax denominator falls out of the PV
matmul.  Output token rows are disjoint per core -> no collective for O.

v3 schedule: one fused software-pipelined stream.  Lead-in computes
K proj j=0,1 (j-granular gathers so attention can start ~35us in),
V proj (+ split V gather), Q proj m=0,1.  Then a 128-step attention
stream (one step per (m, kp)) keeps the scalar engine (exp) saturated;
PE slack in each step is filled with the remaining K projections
(blocks 0-5) and just-in-time Q projections (m+2 per block).  PV lags
LAG steps behind scores/exp and consumes KV chunks in gather-arrival
order (even kp first).  O projection runs as a tail with Wo streamed
in per-nb waves.

RoPE pair-swap is 4 contiguous-partition SBUF->SBUF DMA copies instead
of a permutation matmul: Wq/Wk columns are host-permuted so each 64-row
head-half holds [evens | odds]; bf16 cos/sin tables (half the DMA bytes
of f32) are built to match.  Softmax denominators are inverted with the
fast fp32 reciprocal approximation (~5x faster than nc.vector.reciprocal).

All matmuls bf16 (fp32 is 4x slower on the PE); fp32 PSUM accum.
PSUM budget: proj 2 banks + scores 2x2 banks + PV 2 banks = 8.
"""
import numpy as np
import ml_dtypes

D_MODEL = 2048
KV_DIM = 1024
B = 2
S = 2048
SC = 512            # tokens per core
N_CORES = 8
ROPE_BASE = 10000.0
BF16 = ml_dtypes.bfloat16

_cache = {}


def _host_prep():
    if "perm" in _cache:
        return
    # head-pairing permutation (as v2): row chunk m pairs the two query
    # heads whose KV-head halves share a K chunk.
    perm = np.zeros(D_MODEL, dtype=np.int64)
    for g in range(16):
        for qi in range(2):
            for d in range(64):
                f = g * 128 + qi * 64 + d
                p = ((g // 2) * 2 + qi) * 128 + (g % 2) * 64 + d
                perm[p] = f
    _cache["perm"] = perm  # Wo rows keep this (attnout dims are not eo-split)

    # evens-then-odds split within each 64-row head-half, so the rope
    # pair-swap becomes two contiguous 32-partition block swaps.
    eo = np.concatenate([np.arange(0, 64, 2), np.arange(1, 64, 2)])
    qperm = np.zeros(D_MODEL, dtype=np.int64)
    for b64 in range(D_MODEL // 64):
        qperm[64 * b64:64 * (b64 + 1)] = perm[64 * b64 + eo]
    _cache["qperm"] = qperm
    kperm = np.zeros(KV_DIM, dtype=np.int64)
    for b64 in range(KV_DIM // 64):
        kperm[64 * b64:64 * (b64 + 1)] = 64 * b64 + eo
    _cache["kperm"] = kperm

    theta = ROPE_BASE ** (-np.arange(1024, dtype=np.float64) / 1024.0)
    tabs = []
    for ci in range(4):
        pos = np.arange(ci * SC, (ci + 1) * SC, dtype=np.float64)
        tab = np.zeros((24, 128, 2 * SC), dtype=np.float64)
        for c in range(24):
            if c < 8:
                flat = kperm[np.arange(128 * c, 128 * (c + 1))]
            else:
                flat = qperm[128 * (c - 8):128 * (c - 7)]
            ang = theta[flat // 2][:, None] * pos[None, :]
            sign = np.where(flat % 2 == 0, -1.0, 1.0)
            tab[c, :, :SC] = np.cos(ang)
            tab[c, :, SC:] = sign[:, None] * np.sin(ang)
        tabs.append(tab.astype(BF16))
    _cache["tabs"] = tabs

    # 32-row block-swap permutation (evens block <-> odds block per head-half)
    Pswap = np.zeros((128, 128), dtype=np.float32)
    for h in range(2):
        for b in range(32):
            Pswap[64 * h + 32 + b, 64 * h + b] = 1.0
            Pswap[64 * h + b, 64 * h + 32 + b] = 1.0
    _cache["Pswap"] = Pswap.astype(BF16)


def _build_nc():
    if "nc" in _cache:
        return _cache["nc"]
    import concourse.bacc as bacc
    import concourse.mybir as mybir
    import concourse.tile as tile

    f32 = mybir.dt.float32
    bf16 = mybir.dt.bfloat16
    Exp = mybir.ActivationFunctionType.Exp
    mult = mybir.AluOpType.mult
    add = mybir.AluOpType.add

    nc = bacc.Bacc("TRN2", target_bir_lowering=False, debug=False,
                   num_devices=N_CORES)

    # xT/Wq/Wk are host-tiled so every SBUF load is one contiguous
    # full-rate DMA ([128, 2048] per chunk; the strided/rearranged loads
    # measured ~82GB/s and clogged the queues)
    xT_in = nc.dram_tensor("xT", [128, 16 * SC], bf16, kind="ExternalInput").ap()
    Wq_in = nc.dram_tensor("Wq", [16, 128, D_MODEL], bf16, kind="ExternalInput").ap()
    Wk_in = nc.dram_tensor("Wk", [8, 128, D_MODEL], bf16, kind="ExternalInput").ap()
    Wv_in = nc.dram_tensor("Wv", [D_MODEL, KV_DIM], bf16, kind="ExternalInput").ap()
    Wo_in = nc.dram_tensor("Wo", [D_MODEL, D_MODEL], bf16, kind="ExternalInput").ap()
    rt_in = nc.dram_tensor("ropetab", [24, 128, 2 * SC], bf16,
                           kind="ExternalInput").ap()
    psw_in = nc.dram_tensor("Pswap", [128, 128], bf16, kind="ExternalInput").ap()
    out_dram = nc.dram_tensor("out", [SC, D_MODEL], f32, kind="ExternalOutput").ap()

    GROUPS = [[0, 1, 2, 3], [4, 5, 6, 7]]
    LAG = 8

    from contextlib import ExitStack

    with tile.TileContext(nc) as tc, nc.allow_low_precision(reason="bf16 matmul pipeline by design"):
        with ExitStack() as ostack:
            dram = ostack.enter_context(tc.tile_pool(name="dram", bufs=1, space="DRAM"))
            persist = ostack.enter_context(tc.tile_pool(name="persist", bufs=1))
            kfp = ostack.enter_context(tc.tile_pool(name="kfp", bufs=5))   # Kfull rotation
            vfp = ostack.enter_context(tc.tile_pool(name="vfp", bufs=1))   # Vfull (static)
            wop = ostack.enter_context(tc.tile_pool(name="wop", bufs=2))   # Wv halves + Wo waves
            qrp = ostack.enter_context(tc.tile_pool(name="qrp", bufs=6))   # qr rotation
            k_loc = dram.tile([KV_DIM, SC], bf16, tag="k_loc")
            # V staged per head-half so heads 0-7 can gather early
            v_loc = [dram.tile([SC, 520], bf16, tag=f"v_loc{h}", name=f"v_loc{h}")
                     for h in range(2)]
            k_gat = [dram.tile([4 * 128, SC], bf16, tag=f"kgat{j}",
                               name=f"kgat{j}") for j in range(8)]
            v_gat = [dram.tile([4 * SC, 520], bf16, tag=f"vgat{h}",
                               name=f"vgat{h}") for h in range(2)]

            attnout = [persist.tile([128, SC], bf16, tag=f"ao{m}", name=f"ao{m}")
                       for m in range(16)]
            # Vfull[set][kc]: kt-chunk kc of heads 8*set..8*set+7 (65 cols/head)
            Vfull = [[vfp.tile([128, 520], bf16, tag=f"Vf{h}_{i}",
                               name=f"Vf{h}_{i}") for i in range(16)]
                     for h in range(2)]
            Kfull = {}
            qr = {}

            def fetch_wo_set(nb):
                tiles = [wop.tile([128, 512], bf16, tag=f"wo{mm}",
                                  name=f"wo{nb}_{mm}") for mm in range(16)]
                for mm in range(16):
                    nc.sync.dma_start(
                        out=tiles[mm][:],
                        in_=Wo_in[128 * mm:128 * (mm + 1),
                                  512 * nb:512 * (nb + 1)])
                return tiles

            wo_sets = {}

            with ExitStack() as istack:
                xtp = istack.enter_context(tc.tile_pool(name="xtp", bufs=1))
                wkp = istack.enter_context(tc.tile_pool(name="wkp", bufs=2))
                wqp = istack.enter_context(tc.tile_pool(name="wqp", bufs=2))
                rpool = istack.enter_context(tc.tile_pool(name="rpool", bufs=2))
                rtp = istack.enter_context(tc.tile_pool(name="rtp", bufs=4))
                ppool = istack.enter_context(tc.tile_pool(name="ppool", bufs=2, space="PSUM"))
                spool = istack.enter_context(tc.tile_pool(name="spool", bufs=1, space="PSUM"))
                pvpool = istack.enter_context(tc.tile_pool(name="pvpool", bufs=2, space="PSUM"))
                vastack = ExitStack()
                vapool = vastack.enter_context(tc.tile_pool(name="vapool", bufs=1))
                # ---------- input / weight streams ----------
                xT_all = xtp.tile([128, 16 * SC], bf16, tag="xT", name="xT")
                for h in range(4):
                    eng = nc.sync if h % 2 == 0 else nc.scalar
                    sl = slice(4 * SC * h, 4 * SC * (h + 1))
                    eng.dma_start(out=xT_all[:, sl], in_=xT_in[:, sl])
                def xTc(i, lo=0, hi=SC):
                    return xT_all[:, SC * i + lo:SC * i + hi]

                wkc = {}
                def fetch_wk(j, eng):
                    w = wkp.tile([128, 2048], bf16, tag="wkc", name=f"wkc{j}")
                    eng.dma_start(out=w[:], in_=Wk_in[j])
                    wkc[j] = w

                wqc = {}
                def fetch_wq(m, eng):
                    w = wqp.tile([128, 2048], bf16, tag="wqc", name=f"wqc{m}")
                    eng.dma_start(out=w[:], in_=Wq_in[m])
                    wqc[m] = w

                fetch_wk(0, nc.scalar)
                fetch_wk(1, nc.scalar)

                # prime the Exp table so the first stream activation is fast
                dummy = rpool.tile([1, 64], bf16, tag="dummy")
                nc.vector.memset(dummy[:], 0.0)
                nc.scalar.activation(dummy[:], dummy[:], Exp)

                fetch_wq(0, nc.sync)
                fetch_wq(1, nc.sync)

                rts = {}
                def fetch_rt(chunk, eng):
                    rt = rtp.tile([128, 2 * SC], bf16, tag="rt", name=f"rt{chunk}")
                    eng.dma_start(out=rt[:], in_=rt_in[chunk])
                    rts[chunk] = rt

                fetch_rt(0, nc.sync)
                fetch_rt(1, nc.sync)
                fetch_rt(8, nc.sync)
                fetch_rt(9, nc.sync)

                # V-proj weights early on the scalar queue (pure DMAs first,
                # so later dep-waiting ops never block the queue head).
                # Weight tiles borrow the Wo pool (dead until block 11).
                wv_sets = []
                for nb in range(2):
                    wv = [wop.tile([128, 512], bf16, tag=f"wo{kc}",
                                   name=f"wv{nb}_{kc}") for kc in range(16)]
                    for kc in range(16):
                        nc.scalar.dma_start(
                            out=wv[kc][:],
                            in_=Wv_in[128 * kc:128 * (kc + 1),
                                      512 * nb:512 * (nb + 1)])
                    wv_sets.append(wv)

                psw = rpool.tile([128, 128], bf16, tag="psw", bufs=1)
                nc.sync.dma_start(out=psw[:], in_=psw_in[:])

                def rope(src_psum, dst_bf16, tab_chunk):
                    """dst = src*cos + swap(src)*signsin.  swap exchanges the
                    32-row evens/odds blocks within each 64-row head-half,
                    done by an in-place permutation matmul on the same PSUM
                    tile (qb and t1 are extracted first, so no extra bank)."""
                    rt = rts.pop(tab_chunk)
                    qb = rpool.tile([128, SC], bf16, tag="rope_qb", name="qb")
                    nc.vector.tensor_copy(qb[:], src_psum[:])
                    t1 = rpool.tile([128, SC], f32, tag="rope_t1", name="t1")
                    nc.vector.tensor_tensor(t1[:], src_psum[:], rt[:, 0:SC], mult)
                    nc.tensor.matmul(src_psum[:], psw[:], qb[:],
                                     start=True, stop=True)
                    t2 = rpool.tile([128, SC], bf16, tag="rope_t2", name="t2")
                    nc.vector.tensor_tensor(t2[:], src_psum[:], rt[:, SC:2 * SC], mult)
                    nc.vector.tensor_tensor(dst_bf16[:], t1[:], t2[:], add)

                # ---------- projection emitters ----------
                kfil_ps = {}

                def kproj_mm(j, kc):
                    if kc == 0:
                        kfil_ps[j] = ppool.tile([128, SC], f32, tag="proj",
                                                name=f"psk{j}")
                    nc.tensor.matmul(kfil_ps[j][:], wkc[j][:, 128 * kc:128 * (kc + 1)],
                                     xTc(kc), start=(kc == 0), stop=(kc == 15))

                def kproj_finish(j):
                    del wkc[j]
                    kr = rpool.tile([128, SC], bf16, tag="kr", name=f"kr{j}")
                    rope(kfil_ps.pop(j), kr, j)
                    nc.gpsimd.dma_start(out=k_loc[128 * j:128 * (j + 1), :],
                                        in_=kr[:])

                def kgather(j):
                    Kfull[j] = kfp.tile([128, 4 * SC], bf16, tag="Kf",
                                        name=f"Kf{j}")
                    nc.gpsimd.collective_compute(
                        "AllGather", mybir.AluOpType.bypass,
                        replica_groups=GROUPS,
                        ins=[k_loc[128 * j:128 * (j + 1), :]],
                        outs=[k_gat[j][:]])
                    for c in range(4):
                        nc.gpsimd.dma_start(
                            out=Kfull[j][:, SC * c:SC * (c + 1)],
                            in_=k_gat[j][128 * c:128 * (c + 1), :])

                def kgather_group(j0, nj, gat):
                    # one collective for K chunks j0..j0+nj-1 (fewer, bigger
                    # CC ops: less per-op overhead and less gpsimd blocking)
                    for j in range(j0, j0 + nj):
                        Kfull[j] = kfp.tile([128, 4 * SC], bf16, tag="Kf",
                                            name=f"Kf{j}")
                    nc.gpsimd.collective_compute(
                        "AllGather", mybir.AluOpType.bypass,
                        replica_groups=GROUPS,
                        ins=[k_loc[128 * j0:128 * (j0 + nj), :]],
                        outs=[gat[:]])
                    for j in range(j0, j0 + nj):
                        for c in range(4):
                            nc.gpsimd.dma_start(
                                out=Kfull[j][:, SC * c:SC * (c + 1)],
                                in_=gat[128 * (nj * c + (j - j0)):
                                        128 * (nj * c + (j - j0)) + 128, :])

                qproj_ps = {}

                def qproj_mm(m, kc):
                    if kc == 0:
                        qproj_ps[m] = ppool.tile([128, SC], f32, tag="proj",
                                                 name=f"psq{m}")
                    nc.tensor.matmul(qproj_ps[m][:],
                                     wqc[m][:, 128 * kc:128 * (kc + 1)],
                                     xTc(kc), start=(kc == 0), stop=(kc == 15))

                def qproj_finish(m):
                    qr[m] = qrp.tile([128, SC], bf16, tag="qr", name=f"qr{m}")
                    rope(qproj_ps.pop(m), qr[m], 8 + m)
                    del wqc[m]

                # ---------- lead-in ----------
                va = [vapool.tile([128, 1040], bf16, tag=f"vaug{t}", name=f"vaug{t}")
                      for t in range(4)]
                for t in range(4):
                    nc.vector.memset(va[t][:], 1.0)

                def vproj_half(hh):
                    # V projection for heads 8*hh..8*hh+7 (ones col per head)
                    wv = wv_sets[hh]
                    for t in range(4):
                        ps = ppool.tile([128, SC], f32, tag="proj", name="psv")
                        for kc in range(16):
                            nc.tensor.matmul(
                                ps[:], xTc(kc, 128 * t, 128 * (t + 1)),
                                wv[kc][:],
                                start=(kc == 0), stop=(kc == 15))
                        dst = va[t][:, 520 * hh:520 * (hh + 1)].rearrange(
                            "p (h d) -> p h d", h=8)[:, :, 0:64]
                        src = ps[:].rearrange("p (h d) -> p h d", h=8)
                        nc.vector.tensor_copy(dst, src)
                    for t in range(4):
                        nc.scalar.dma_start(
                            out=v_loc[hh][128 * t:128 * (t + 1), :],
                            in_=va[t][:, 520 * hh:520 * (hh + 1)])

                def vgather(hh):
                    nc.gpsimd.collective_compute(
                        "AllGather", mybir.AluOpType.bypass, replica_groups=GROUPS,
                        ins=[v_loc[hh][:]], outs=[v_gat[hh][:]])
                    for i in range(16):
                        nc.gpsimd.dma_start(
                            out=Vfull[hh][i][:],
                            in_=v_gat[hh][128 * i:128 * (i + 1), :])

                # PE order interleaves K/V/Q so the CC stream (g0 | vA | g1 | vB)
                # gets its inputs early while qr0 also lands early
                for kc in range(16):
                    kproj_mm(0, kc)
                kproj_finish(0)
                kgather(0)
                vproj_half(0)
                vgather(0)
                fetch_wq(2, nc.sync)
                for kc in range(16):
                    qproj_mm(0, kc)
                qproj_finish(0)
                for kc in range(16):
                    kproj_mm(1, kc)
                kproj_finish(1)
                kgather(1)
                vproj_half(1)
                vgather(1)
                for kc in range(16):
                    qproj_mm(1, kc)
                qproj_finish(1)
                # va/wv space is dead from here; let epool/npool reuse it
                vastack.close()
                epool = istack.enter_context(tc.tile_pool(name="epool", bufs=16))
                npool = istack.enter_context(tc.tile_pool(name="npool", bufs=2))

                # ---------- attention stream ----------
                e_tiles = {}

                def emit_scores(m, kp):
                    j = m // 2
                    spA = spool.tile([128, 1024], f32, tag="spA", name="spA")
                    spB = spool.tile([128, 1024], f32, tag="spB", name="spB")
                    sp = [spB, spA]  # half0 -> B, half1 -> A
                    for u in range(2):
                        for half in (1, 0):
                            kc = 2 * kp + u
                            nc.tensor.matmul(
                                sp[half][:, 512 * u:512 * (u + 1)],
                                Kfull[j][64 * half:64 * (half + 1),
                                         128 * kc:128 * (kc + 1)],
                                qr[m][64 * half:64 * (half + 1), :],
                                start=True, stop=True)
                    e = [None, None]
                    for half in (1, 0):
                        et = epool.tile([128, 1024], bf16, tag="exp", name="et")
                        nc.scalar.activation(et[:], sp[half][:], Exp, scale=0.125)
                        e[half] = et
                    e_tiles[(m, kp)] = e

                pv_of = {}

                pending_norms = []

                def emit_norm(m, pv):
                    # part 1: drain pv + reciprocal; the gpsimd broadcast and
                    # final mult are deferred a few steps so a broadcast stuck
                    # behind a collective never blocks the DVE queue head
                    recs, raws = [], []
                    for half in range(2):
                        dex = npool.tile([1, SC], f32, tag="dex", bufs=2, name="dex")
                        nc.vector.tensor_copy(dex[:], pv[half][64:65, :])
                        raw = npool.tile([65, SC], bf16, tag="raw", bufs=4, name="raw")
                        nc.vector.tensor_copy(raw[:], pv[half][:])
                        rec = npool.tile([1, SC], f32, tag="rec", bufs=2, name="rec")
                        nc.vector.reciprocal_approx_fast(out=rec[:], in_=dex[:])
                        bcs = npool.tile([64, SC], f32, tag="bcs", bufs=2, name="bcs")
                        nc.gpsimd.partition_broadcast(bcs[:], rec[:], channels=64)
                        recs.append(bcs)
                        raws.append(raw)
                    pending_norms.append((m, recs, raws))

                def norm_part2():
                    m, bcss, raws = pending_norms.pop(0)
                    for half in range(2):
                        nc.vector.tensor_tensor(
                            attnout[m][64 * half:64 * (half + 1), :],
                            raws[half][0:64, :], bcss[half][:], mult)

                def emit_pv(s2):
                    pm, kp = s2 // 8, s2 % 8
                    if kp == 0:
                        pv_of[pm] = [pvpool.tile([65, SC], f32, tag="pv",
                                                 name="pv") for _ in range(2)]
                    pv = pv_of[pm]
                    e = e_tiles.pop((pm, kp))
                    j = pm // 2
                    for half in range(2):
                        g = 2 * j + half
                        hs, gc = g // 8, g % 8
                        for u in range(2):
                            kc = 2 * kp + u
                            nc.tensor.matmul(
                                pv[half][:],
                                Vfull[hs][kc][:, 65 * gc:65 * (gc + 1)],
                                e[half][:, 512 * u:512 * (u + 1)],
                                start=(kp == 0 and u == 0),
                                stop=(kp == 7 and u == 1))
                    if kp == 7:
                        emit_norm(pm, pv_of.pop(pm))

                for s in range(128):
                    m, kp = s // 8, s % 8
                    # PV (lagged)
                    if s >= LAG:
                        emit_pv(s - LAG)
                    if pending_norms and s >= 8 * pending_norms[0][0] + 13 + 6:
                        norm_part2()
                    # K-proj filler in blocks 0-5 (j = 2..7)
                    if m < 6:
                        jf = 2 + m
                        if kp == 0:
                            fetch_wk(jf, nc.sync)
                            fetch_rt(jf, nc.sync)
                        for kc in (2 * kp, 2 * kp + 1):
                            kproj_mm(jf, kc)
                        if kp == 7:
                            kproj_finish(jf)
                            kgather(jf)
                    # Q-proj filler (m+2), 2 matmuls per step
                    mq = m + 2
                    if mq < 16:
                        if kp == 0:
                            if mq + 1 < 16:
                                fetch_wq(mq + 1, nc.sync)
                            fetch_rt(8 + mq, nc.sync)
                        for kc in (2 * kp, 2 * kp + 1):
                            qproj_mm(mq, kc)
                        if kp == 7:
                            qproj_finish(mq)
                    # Wo prefetch waves late in the stream
                    if m == 11 and kp == 0:
                        wo_sets[0] = fetch_wo_set(0)
                    if m == 13 and kp == 0:
                        wo_sets[1] = fetch_wo_set(1)
                    # scores + exp for this step
                    emit_scores(m, kp)

                # drain remaining PV groups + norms
                for s2 in range(128 - LAG, 128):
                    emit_pv(s2)
                while pending_norms:
                    norm_part2()

            # ---------- O projection tail ----------
            with ExitStack() as tstack:
                opsum = tstack.enter_context(tc.tile_pool(name="opsum", bufs=2, space="PSUM"))
                ostage = tstack.enter_context(tc.tile_pool(name="ostage", bufs=4))
                wo_sets[2] = fetch_wo_set(2)
                for nb in range(4):
                    if nb == 1:
                        wo_sets[3] = fetch_wo_set(3)
                    wset = wo_sets[nb]
                    for t in range(4):
                        ps = opsum.tile([128, 512], f32, tag="ops")
                        for mm in range(16):
                            nc.tensor.matmul(
                                ps[:],
                                attnout[mm][:, 128 * t:128 * (t + 1)],
                                wset[mm][:],
                                start=(mm == 0), stop=(mm == 15))
                        ot = ostage.tile([128, 512], f32, tag="ot")
                        nc.scalar.copy(ot[:], ps[:])
                        eng = nc.gpsimd if (t % 2 == 0) else nc.scalar
                        eng.dma_start(
                            out=out_dram[128 * t:128 * (t + 1),
                                         512 * nb:512 * (nb + 1)],
                            in_=ot[:])

    nc.compile()
    _cache["nc"] = nc
    return nc


def kernel(x, Wq, Wk, Wv, Wo):
    from concourse.bass_utils import run_bass_kernel_spmd

    _host_prep()
    x = np.asarray(x, dtype=np.float32)
    qperm = _cache["qperm"]
    kperm = _cache["kperm"]
    perm = _cache["perm"]
    Wq_perm = np.asarray(Wq, dtype=np.float32)[:, qperm].astype(BF16)
    Wk_perm = np.asarray(Wk, dtype=np.float32)[:, kperm].astype(BF16)
    # tile for contiguous per-chunk loads: W_t[chunk, p, kc*128+c] = W[128*kc+p, 128*chunk+c]
    Wq_t = np.ascontiguousarray(
        Wq_perm.reshape(16, 128, 16, 128).transpose(2, 1, 0, 3).reshape(16, 128, D_MODEL))
    Wk_t = np.ascontiguousarray(
        Wk_perm.reshape(16, 128, 8, 128).transpose(2, 1, 0, 3).reshape(8, 128, D_MODEL))
    Wv_b = np.asarray(Wv, dtype=np.float32).astype(BF16)
    Wo_perm = np.ascontiguousarray(np.asarray(Wo, dtype=np.float32)[perm, :]).astype(BF16)

    in_maps = []
    for core in range(N_CORES):
        b, ci = core // 4, core % 4
        xT = np.ascontiguousarray(x[b, ci * SC:(ci + 1) * SC, :].T).astype(BF16)
        xT_t = np.ascontiguousarray(
            xT.reshape(16, 128, SC).transpose(1, 0, 2).reshape(128, 16 * SC))
        in_maps.append({
            "xT": xT_t, "Wq": Wq_t, "Wk": Wk_t, "Wv": Wv_b, "Wo": Wo_perm,
            "ropetab": _cache["tabs"][ci], "Pswap": _cache["Pswap"],
        })
    _cache["in_maps"] = in_maps

    nc = _build_nc()
    res = run_bass_kernel_spmd(nc, in_maps, list(range(N_CORES)))
    out = np.zeros((B, S, D_MODEL), dtype=np.float32)
    for core in range(N_CORES):
        b, ci = core // 4, core % 4
        out[b, ci * SC:(ci + 1) * SC, :] = res.results[core]["out"]
    return out


# revision 54
# speedup vs baseline: 1.0797x; 1.0557x over previous
"""GQA attention kernel for 8 TRN2 NeuronCores (Bass/Tile) — v3.

Sharding: tokens sharded 8 ways (2 batches x 4 chunks of 512).  Each core
computes Q/K/V projections for its 512 tokens in a transposed
(feature-on-partition) layout, all-gathers K/V within its 4-core batch
group, then runs attention with scores in [k_tok, q_tok] layout and a
ones column appended to V so the softmax denominator falls out of the PV
matmul.  Output token rows are disjoint per core -> no collective for O.

v3 schedule: one fused software-pipelined stream.  Lead-in computes
K proj j=0,1 (j-granular gathers so attention can start ~35us in),
V proj (+ split V gather), Q proj m=0,1.  Then a 128-step attention
stream (one step per (m, kp)) keeps the scalar engine (exp) saturated;
PE slack in each step is filled with the remaining K projections
(blocks 0-5) and just-in-time Q projections (m+2 per block).  PV lags
LAG steps behind scores/exp and consumes KV chunks in gather-arrival
order (even kp first).  O projection runs as a tail with Wo streamed
in per-nb waves.

RoPE pair-swap is 4 contiguous-partition SBUF->SBUF DMA copies instead
of a permutation matmul: Wq/Wk columns are host-permuted so each 64-row
head-half holds [evens | odds]; bf16 cos/sin tables (half the DMA bytes
of f32) are built to match.  Softmax denominators are inverted with the
fast fp32 reciprocal approximation (~5x faster than nc.vector.reciprocal).

All matmuls bf16 (fp32 is 4x slower on the PE); fp32 PSUM accum.
PSUM budget: proj 2 banks + scores 2x2 banks + PV 2 banks = 8.
"""
import numpy as np
import ml_dtypes

D_MODEL = 2048
KV_DIM = 1024
B = 2
S = 2048
SC = 512            # tokens per core
N_CORES = 8
ROPE_BASE = 10000.0
BF16 = ml_dtypes.bfloat16

_cache = {}


def _host_prep():
    if "perm" in _cache:
        return
    # head-pairing permutation (as v2): row chunk m pairs the two query
    # heads whose KV-head halves share a K chunk.
    perm = np.zeros(D_MODEL, dtype=np.int64)
    for g in range(16):
        for qi in range(2):
            for d in range(64):
                f = g * 128 + qi * 64 + d
                p = ((g // 2) * 2 + qi) * 128 + (g % 2) * 64 + d
                perm[p] = f
    _cache["perm"] = perm  # Wo rows keep this (attnout dims are not eo-split)

    # evens-then-odds split within each 64-row head-half, so the rope
    # pair-swap becomes two contiguous 32-partition block swaps.
    eo = np.concatenate([np.arange(0, 64, 2), np.arange(1, 64, 2)])
    qperm = np.zeros(D_MODEL, dtype=np.int64)
    for b64 in range(D_MODEL // 64):
        qperm[64 * b64:64 * (b64 + 1)] = perm[64 * b64 + eo]
    _cache["qperm"] = qperm
    kperm = np.zeros(KV_DIM, dtype=np.int64)
    for b64 in range(KV_DIM // 64):
        kperm[64 * b64:64 * (b64 + 1)] = 64 * b64 + eo
    _cache["kperm"] = kperm

    theta = ROPE_BASE ** (-np.arange(1024, dtype=np.float64) / 1024.0)
    tabs = []
    for ci in range(4):
        pos = np.arange(ci * SC, (ci + 1) * SC, dtype=np.float64)
        tab = np.zeros((24, 128, 2 * SC), dtype=np.float64)
        for c in range(24):
            if c < 8:
                flat = kperm[np.arange(128 * c, 128 * (c + 1))]
            else:
                flat = qperm[128 * (c - 8):128 * (c - 7)]
            ang = theta[flat // 2][:, None] * pos[None, :]
            sign = np.where(flat % 2 == 0, -1.0, 1.0)
            tab[c, :, :SC] = np.cos(ang)
            tab[c, :, SC:] = sign[:, None] * np.sin(ang)
        tabs.append(tab.astype(BF16))
    _cache["tabs"] = tabs

    # 32-row block-swap permutation (evens block <-> odds block per head-half)
    Pswap = np.zeros((128, 128), dtype=np.float32)
    for h in range(2):
        for b in range(32):
            Pswap[64 * h + 32 + b, 64 * h + b] = 1.0
            Pswap[64 * h + b, 64 * h + 32 + b] = 1.0
    _cache["Pswap"] = Pswap.astype(BF16)


def _build_nc():
    if "nc" in _cache:
        return _cache["nc"]
    import concourse.bacc as bacc
    import concourse.mybir as mybir
    import concourse.tile as tile

    f32 = mybir.dt.float32
    bf16 = mybir.dt.bfloat16
    Exp = mybir.ActivationFunctionType.Exp
    mult = mybir.AluOpType.mult
    add = mybir.AluOpType.add

    nc = bacc.Bacc("TRN2", target_bir_lowering=False, debug=False,
                   num_devices=N_CORES)

    # xT/Wq/Wk are host-tiled so every SBUF load is one contiguous
    # full-rate DMA ([128, 2048] per chunk; the strided/rearranged loads
    # measured ~82GB/s and clogged the queues)
    xT_in = nc.dram_tensor("xT", [128, 16 * SC], bf16, kind="ExternalInput").ap()
    Wq_in = nc.dram_tensor("Wq", [16, 128, D_MODEL], bf16, kind="ExternalInput").ap()
    Wk_in = nc.dram_tensor("Wk", [8, 128, D_MODEL], bf16, kind="ExternalInput").ap()
    Wv_in = nc.dram_tensor("Wv", [D_MODEL, KV_DIM], bf16, kind="ExternalInput").ap()
    Wo_in = nc.dram_tensor("Wo", [D_MODEL, D_MODEL], bf16, kind="ExternalInput").ap()
    rt_in = nc.dram_tensor("ropetab", [24, 128, 2 * SC], bf16,
                           kind="ExternalInput").ap()
    psw_in = nc.dram_tensor("Pswap", [128, 128], bf16, kind="ExternalInput").ap()
    out_dram = nc.dram_tensor("out", [SC, D_MODEL], f32, kind="ExternalOutput").ap()

    GROUPS = [[0, 1, 2, 3], [4, 5, 6, 7]]
    LAG = 6

    from contextlib import ExitStack

    with tile.TileContext(nc) as tc, nc.allow_low_precision(reason="bf16 matmul pipeline by design"):
        with ExitStack() as ostack:
            dram = ostack.enter_context(tc.tile_pool(name="dram", bufs=1, space="DRAM"))
            persist = ostack.enter_context(tc.tile_pool(name="persist", bufs=1))
            kfp = ostack.enter_context(tc.tile_pool(name="kfp", bufs=5))   # Kfull rotation
            vfp = ostack.enter_context(tc.tile_pool(name="vfp", bufs=1))   # Vfull (static)
            wop = ostack.enter_context(tc.tile_pool(name="wop", bufs=2))   # Wv halves + Wo waves
            qrp = ostack.enter_context(tc.tile_pool(name="qrp", bufs=6))   # qr rotation
            k_loc = dram.tile([KV_DIM, SC], bf16, tag="k_loc")
            # V staged per head-half so heads 0-7 can gather early
            v_loc = [dram.tile([SC, 520], bf16, tag=f"v_loc{h}", name=f"v_loc{h}")
                     for h in range(2)]
            k_gat = [dram.tile([4 * 128, SC], bf16, tag=f"kgat{j}",
                               name=f"kgat{j}") for j in range(8)]
            v_gat = [dram.tile([4 * SC, 520], bf16, tag=f"vgat{h}",
                               name=f"vgat{h}") for h in range(2)]

            attnout = [persist.tile([128, SC], bf16, tag=f"ao{m}", name=f"ao{m}")
                       for m in range(16)]
            # Vfull[set][kc]: kt-chunk kc of heads 8*set..8*set+7 (65 cols/head)
            Vfull = [[vfp.tile([128, 520], bf16, tag=f"Vf{h}_{i}",
                               name=f"Vf{h}_{i}") for i in range(16)]
                     for h in range(2)]
            Kfull = {}
            qr = {}

            def fetch_wo_set(nb):
                tiles = [wop.tile([128, 512], bf16, tag=f"wo{mm}",
                                  name=f"wo{nb}_{mm}") for mm in range(16)]
                for mm in range(16):
                    nc.sync.dma_start(
                        out=tiles[mm][:],
                        in_=Wo_in[128 * mm:128 * (mm + 1),
                                  512 * nb:512 * (nb + 1)])
                return tiles

            wo_sets = {}

            with ExitStack() as istack:
                xtp = istack.enter_context(tc.tile_pool(name="xtp", bufs=1))
                wkp = istack.enter_context(tc.tile_pool(name="wkp", bufs=2))
                wqp = istack.enter_context(tc.tile_pool(name="wqp", bufs=2))
                rpool = istack.enter_context(tc.tile_pool(name="rpool", bufs=2))
                rtp = istack.enter_context(tc.tile_pool(name="rtp", bufs=4))
                ppool = istack.enter_context(tc.tile_pool(name="ppool", bufs=2, space="PSUM"))
                spool = istack.enter_context(tc.tile_pool(name="spool", bufs=1, space="PSUM"))
                pvpool = istack.enter_context(tc.tile_pool(name="pvpool", bufs=2, space="PSUM"))
                vastack = ExitStack()
                vapool = vastack.enter_context(tc.tile_pool(name="vapool", bufs=1))
                # ---------- input / weight streams ----------
                xT_all = xtp.tile([128, 16 * SC], bf16, tag="xT", name="xT")
                for h in range(4):
                    eng = nc.sync if h % 2 == 0 else nc.scalar
                    sl = slice(4 * SC * h, 4 * SC * (h + 1))
                    eng.dma_start(out=xT_all[:, sl], in_=xT_in[:, sl])
                def xTc(i, lo=0, hi=SC):
                    return xT_all[:, SC * i + lo:SC * i + hi]

                wkc = {}
                def fetch_wk(j, eng):
                    w = wkp.tile([128, 2048], bf16, tag="wkc", name=f"wkc{j}")
                    eng.dma_start(out=w[:], in_=Wk_in[j])
                    wkc[j] = w

                wqc = {}
                def fetch_wq(m, eng):
                    w = wqp.tile([128, 2048], bf16, tag="wqc", name=f"wqc{m}")
                    eng.dma_start(out=w[:], in_=Wq_in[m])
                    wqc[m] = w

                fetch_wk(0, nc.scalar)
                fetch_wk(1, nc.scalar)

                # prime the Exp table so the first stream activation is fast
                dummy = rpool.tile([1, 64], bf16, tag="dummy")
                nc.vector.memset(dummy[:], 0.0)
                nc.scalar.activation(dummy[:], dummy[:], Exp)

                fetch_wq(0, nc.sync)
                fetch_wq(1, nc.sync)

                rts = {}
                def fetch_rt(chunk, eng):
                    rt = rtp.tile([128, 2 * SC], bf16, tag="rt", name=f"rt{chunk}")
                    eng.dma_start(out=rt[:], in_=rt_in[chunk])
                    rts[chunk] = rt

                fetch_rt(0, nc.sync)
                fetch_rt(1, nc.sync)
                fetch_rt(8, nc.sync)
                fetch_rt(9, nc.sync)

                # V-proj weights early on the scalar queue (pure DMAs first,
                # so later dep-waiting ops never block the queue head).
                # Weight tiles borrow the Wo pool (dead until block 11).
                wv_sets = []
                for nb in range(2):
                    wv = [wop.tile([128, 512], bf16, tag=f"wo{kc}",
                                   name=f"wv{nb}_{kc}") for kc in range(16)]
                    for kc in range(16):
                        nc.scalar.dma_start(
                            out=wv[kc][:],
                            in_=Wv_in[128 * kc:128 * (kc + 1),
                                      512 * nb:512 * (nb + 1)])
                    wv_sets.append(wv)

                psw = rpool.tile([128, 128], bf16, tag="psw", bufs=1)
                nc.sync.dma_start(out=psw[:], in_=psw_in[:])

                def rope(src_psum, dst_bf16, tab_chunk):
                    """dst = src*cos + swap(src)*signsin.  swap exchanges the
                    32-row evens/odds blocks within each 64-row head-half,
                    done by an in-place permutation matmul on the same PSUM
                    tile (qb and t1 are extracted first, so no extra bank)."""
                    rt = rts.pop(tab_chunk)
                    qb = rpool.tile([128, SC], bf16, tag="rope_qb", name="qb")
                    nc.vector.tensor_copy(qb[:], src_psum[:])
                    t1 = rpool.tile([128, SC], f32, tag="rope_t1", name="t1")
                    nc.vector.tensor_tensor(t1[:], src_psum[:], rt[:, 0:SC], mult)
                    nc.tensor.matmul(src_psum[:], psw[:], qb[:],
                                     start=True, stop=True)
                    t2 = rpool.tile([128, SC], bf16, tag="rope_t2", name="t2")
                    nc.vector.tensor_tensor(t2[:], src_psum[:], rt[:, SC:2 * SC], mult)
                    nc.vector.tensor_tensor(dst_bf16[:], t1[:], t2[:], add)

                # ---------- projection emitters ----------
                kfil_ps = {}

                def kproj_mm(j, kc):
                    if kc == 0:
                        kfil_ps[j] = ppool.tile([128, SC], f32, tag="proj",
                                                name=f"psk{j}")
                    nc.tensor.matmul(kfil_ps[j][:], wkc[j][:, 128 * kc:128 * (kc + 1)],
                                     xTc(kc), start=(kc == 0), stop=(kc == 15))

                def kproj_finish(j):
                    del wkc[j]
                    kr = rpool.tile([128, SC], bf16, tag="kr", name=f"kr{j}")
                    rope(kfil_ps.pop(j), kr, j)
                    nc.gpsimd.dma_start(out=k_loc[128 * j:128 * (j + 1), :],
                                        in_=kr[:])

                def kgather(j):
                    Kfull[j] = kfp.tile([128, 4 * SC], bf16, tag="Kf",
                                        name=f"Kf{j}")
                    nc.gpsimd.collective_compute(
                        "AllGather", mybir.AluOpType.bypass,
                        replica_groups=GROUPS,
                        ins=[k_loc[128 * j:128 * (j + 1), :]],
                        outs=[k_gat[j][:]])
                    for c in range(4):
                        nc.gpsimd.dma_start(
                            out=Kfull[j][:, SC * c:SC * (c + 1)],
                            in_=k_gat[j][128 * c:128 * (c + 1), :])

                def kgather_group(j0, nj, gat):
                    # one collective for K chunks j0..j0+nj-1 (fewer, bigger
                    # CC ops: less per-op overhead and less gpsimd blocking)
                    for j in range(j0, j0 + nj):
                        Kfull[j] = kfp.tile([128, 4 * SC], bf16, tag="Kf",
                                            name=f"Kf{j}")
                    nc.gpsimd.collective_compute(
                        "AllGather", mybir.AluOpType.bypass,
                        replica_groups=GROUPS,
                        ins=[k_loc[128 * j0:128 * (j0 + nj), :]],
                        outs=[gat[:]])
                    for j in range(j0, j0 + nj):
                        for c in range(4):
                            nc.gpsimd.dma_start(
                                out=Kfull[j][:, SC * c:SC * (c + 1)],
                                in_=gat[128 * (nj * c + (j - j0)):
                                        128 * (nj * c + (j - j0)) + 128, :])

                qproj_ps = {}

                def qproj_mm(m, kc):
                    if kc == 0:
                        qproj_ps[m] = ppool.tile([128, SC], f32, tag="proj",
                                                 name=f"psq{m}")
                    nc.tensor.matmul(qproj_ps[m][:],
                                     wqc[m][:, 128 * kc:128 * (kc + 1)],
                                     xTc(kc), start=(kc == 0), stop=(kc == 15))

                def qproj_finish(m):
                    qr[m] = qrp.tile([128, SC], bf16, tag="qr", name=f"qr{m}")
                    rope(qproj_ps.pop(m), qr[m], 8 + m)
                    del wqc[m]

                # ---------- lead-in ----------
                va = [vapool.tile([128, 1040], bf16, tag=f"vaug{t}", name=f"vaug{t}")
                      for t in range(4)]
                for t in range(4):
                    nc.vector.memset(va[t][:], 1.0)

                def vproj_half(hh):
                    # V projection for heads 8*hh..8*hh+7 (ones col per head)
                    wv = wv_sets[hh]
                    for t in range(4):
                        ps = ppool.tile([128, SC], f32, tag="proj", name="psv")
                        for kc in range(16):
                            nc.tensor.matmul(
                                ps[:], xTc(kc, 128 * t, 128 * (t + 1)),
                                wv[kc][:],
                                start=(kc == 0), stop=(kc == 15))
                        dst = va[t][:, 520 * hh:520 * (hh + 1)].rearrange(
                            "p (h d) -> p h d", h=8)[:, :, 0:64]
                        src = ps[:].rearrange("p (h d) -> p h d", h=8)
                        nc.vector.tensor_copy(dst, src)
                    for t in range(4):
                        nc.scalar.dma_start(
                            out=v_loc[hh][128 * t:128 * (t + 1), :],
                            in_=va[t][:, 520 * hh:520 * (hh + 1)])

                def vgather(hh):
                    nc.gpsimd.collective_compute(
                        "AllGather", mybir.AluOpType.bypass, replica_groups=GROUPS,
                        ins=[v_loc[hh][:]], outs=[v_gat[hh][:]])
                    for i in range(16):
                        nc.gpsimd.dma_start(
                            out=Vfull[hh][i][:],
                            in_=v_gat[hh][128 * i:128 * (i + 1), :])

                # PE order interleaves K/V/Q so the CC stream (g0 | vA | g1 | vB)
                # gets its inputs early while qr0 also lands early
                for kc in range(16):
                    kproj_mm(0, kc)
                kproj_finish(0)
                kgather(0)
                vproj_half(0)
                vgather(0)
                fetch_wq(2, nc.sync)
                for kc in range(16):
                    qproj_mm(0, kc)
                qproj_finish(0)
                for kc in range(16):
                    kproj_mm(1, kc)
                kproj_finish(1)
                kgather(1)
                vproj_half(1)
                vgather(1)
                for kc in range(16):
                    qproj_mm(1, kc)
                qproj_finish(1)
                # va/wv space is dead from here; let epool/npool reuse it
                vastack.close()
                epool = istack.enter_context(tc.tile_pool(name="epool", bufs=14))
                npool = istack.enter_context(tc.tile_pool(name="npool", bufs=2))

                # ---------- attention stream ----------
                e_tiles = {}

                def emit_scores(m, kp):
                    j = m // 2
                    spA = spool.tile([128, 1024], f32, tag="spA", name="spA")
                    spB = spool.tile([128, 1024], f32, tag="spB", name="spB")
                    sp = [spB, spA]  # half0 -> B, half1 -> A
                    for u in range(2):
                        for half in (1, 0):
                            kc = 2 * kp + u
                            nc.tensor.matmul(
                                sp[half][:, 512 * u:512 * (u + 1)],
                                Kfull[j][64 * half:64 * (half + 1),
                                         128 * kc:128 * (kc + 1)],
                                qr[m][64 * half:64 * (half + 1), :],
                                start=True, stop=True)
                    e = [None, None]
                    for half in (1, 0):
                        et = epool.tile([128, 1024], bf16, tag="exp", name="et")
                        nc.scalar.activation(et[:], sp[half][:], Exp, scale=0.125)
                        e[half] = et
                    e_tiles[(m, kp)] = e

                pv_of = {}

                pending_norms = []

                def emit_norm(m, pv):
                    # part 1: drain pv + reciprocal; the gpsimd broadcast and
                    # final mult are deferred a few steps so a broadcast stuck
                    # behind a collective never blocks the DVE queue head
                    recs, raws = [], []
                    for half in range(2):
                        dex = npool.tile([1, SC], f32, tag="dex", bufs=2, name="dex")
                        nc.vector.tensor_copy(dex[:], pv[half][64:65, :])
                        raw = npool.tile([65, SC], bf16, tag="raw", bufs=4, name="raw")
                        nc.vector.tensor_copy(raw[:], pv[half][:])
                        rec = npool.tile([1, SC], f32, tag="rec", bufs=2, name="rec")
                        nc.vector.reciprocal_approx_fast(out=rec[:], in_=dex[:])
                        bcs = npool.tile([64, SC], f32, tag="bcs", bufs=2, name="bcs")
                        nc.gpsimd.partition_broadcast(bcs[:], rec[:], channels=64)
                        recs.append(bcs)
                        raws.append(raw)
                    pending_norms.append((m, recs, raws))

                def norm_part2():
                    m, bcss, raws = pending_norms.pop(0)
                    for half in range(2):
                        nc.vector.tensor_tensor(
                            attnout[m][64 * half:64 * (half + 1), :],
                            raws[half][0:64, :], bcss[half][:], mult)

                def emit_pv(s2):
                    pm, kp = s2 // 8, s2 % 8
                    if kp == 0:
                        pv_of[pm] = [pvpool.tile([65, SC], f32, tag="pv",
                                                 name="pv") for _ in range(2)]
                    pv = pv_of[pm]
                    e = e_tiles.pop((pm, kp))
                    j = pm // 2
                    for half in range(2):
                        g = 2 * j + half
                        hs, gc = g // 8, g % 8
                        for u in range(2):
                            kc = 2 * kp + u
                            nc.tensor.matmul(
                                pv[half][:],
                                Vfull[hs][kc][:, 65 * gc:65 * (gc + 1)],
                                e[half][:, 512 * u:512 * (u + 1)],
                                start=(kp == 0 and u == 0),
                                stop=(kp == 7 and u == 1))
                    if kp == 7:
                        emit_norm(pm, pv_of.pop(pm))

                for s in range(128):
                    m, kp = s // 8, s % 8
                    # PV (lagged)
                    if s >= LAG:
                        emit_pv(s - LAG)
                    if pending_norms and s >= 8 * pending_norms[0][0] + 13 + 6:
                        norm_part2()
                    # K-proj filler in blocks 0-5 (j = 2..7)
                    if m < 6:
                        jf = 2 + m
                        if kp == 0:
                            fetch_wk(jf, nc.sync)
                            fetch_rt(jf, nc.sync)
                        for kc in (2 * kp, 2 * kp + 1):
                            kproj_mm(jf, kc)
                        if kp == 7:
                            kproj_finish(jf)
                            kgather(jf)
                    # Q-proj filler (m+2), 2 matmuls per step
                    mq = m + 2
                    if mq < 16:
                        if kp == 0:
                            if mq + 1 < 16:
                                fetch_wq(mq + 1, nc.sync)
                            fetch_rt(8 + mq, nc.sync)
                        for kc in (2 * kp, 2 * kp + 1):
                            qproj_mm(mq, kc)
                        if kp == 7:
                            qproj_finish(mq)
                    # Wo prefetch waves late in the stream
                    if m == 11 and kp == 0:
                        wo_sets[0] = fetch_wo_set(0)
                    if m == 13 and kp == 0:
                        wo_sets[1] = fetch_wo_set(1)
                    # scores + exp for this step
                    emit_scores(m, kp)

                # drain remaining PV groups + norms
                for s2 in range(128 - LAG, 128):
                    emit_pv(s2)
                while pending_norms:
                    norm_part2()

            # ---------- O projection tail ----------
            with ExitStack() as tstack:
                opsum = tstack.enter_context(tc.tile_pool(name="opsum", bufs=2, space="PSUM"))
                ostage = tstack.enter_context(tc.tile_pool(name="ostage", bufs=4))
                wo_sets[2] = fetch_wo_set(2)
                for nb in range(4):
                    if nb == 1:
                        wo_sets[3] = fetch_wo_set(3)
                    wset = wo_sets[nb]
                    for t in range(4):
                        ps = opsum.tile([128, 512], f32, tag="ops")
                        for mm in range(16):
                            nc.tensor.matmul(
                                ps[:],
                                attnout[mm][:, 128 * t:128 * (t + 1)],
                                wset[mm][:],
                                start=(mm == 0), stop=(mm == 15))
                        ot = ostage.tile([128, 512], f32, tag="ot")
                        nc.scalar.copy(ot[:], ps[:])
                        eng = nc.gpsimd if (t % 2 == 0) else nc.scalar
                        eng.dma_start(
                            out=out_dram[128 * t:128 * (t + 1),
                                         512 * nb:512 * (nb + 1)],
                            in_=ot[:])

    nc.compile()
    _cache["nc"] = nc
    return nc


def kernel(x, Wq, Wk, Wv, Wo):
    from concourse.bass_utils import run_bass_kernel_spmd

    _host_prep()
    x = np.asarray(x, dtype=np.float32)
    qperm = _cache["qperm"]
    kperm = _cache["kperm"]
    perm = _cache["perm"]
    Wq_perm = np.asarray(Wq, dtype=np.float32)[:, qperm].astype(BF16)
    Wk_perm = np.asarray(Wk, dtype=np.float32)[:, kperm].astype(BF16)
    # tile for contiguous per-chunk loads: W_t[chunk, p, kc*128+c] = W[128*kc+p, 128*chunk+c]
    Wq_t = np.ascontiguousarray(
        Wq_perm.reshape(16, 128, 16, 128).transpose(2, 1, 0, 3).reshape(16, 128, D_MODEL))
    Wk_t = np.ascontiguousarray(
        Wk_perm.reshape(16, 128, 8, 128).transpose(2, 1, 0, 3).reshape(8, 128, D_MODEL))
    Wv_b = np.asarray(Wv, dtype=np.float32).astype(BF16)
    Wo_perm = np.ascontiguousarray(np.asarray(Wo, dtype=np.float32)[perm, :]).astype(BF16)

    in_maps = []
    for core in range(N_CORES):
        b, ci = core // 4, core % 4
        xT = np.ascontiguousarray(x[b, ci * SC:(ci + 1) * SC, :].T).astype(BF16)
        xT_t = np.ascontiguousarray(
            xT.reshape(16, 128, SC).transpose(1, 0, 2).reshape(128, 16 * SC))
        in_maps.append({
            "xT": xT_t, "Wq": Wq_t, "Wk": Wk_t, "Wv": Wv_b, "Wo": Wo_perm,
            "ropetab": _cache["tabs"][ci], "Pswap": _cache["Pswap"],
        })
    _cache["in_maps"] = in_maps

    nc = _build_nc()
    res = run_bass_kernel_spmd(nc, in_maps, list(range(N_CORES)))
    out = np.zeros((B, S, D_MODEL), dtype=np.float32)
    for core in range(N_CORES):
        b, ci = core // 4, core % 4
        out[b, ci * SC:(ci + 1) * SC, :] = res.results[core]["out"]
    return out
